# revision 12
# baseline (speedup 1.0000x reference)
"""Trainium2 Bass kernel for nn_DynamicGCNModel (2-layer GCN+GRU, 50k nodes,
1.6M edges, C=128) on 8 NeuronCores.

v2 design:
- Nodes sharded 6272/core; edges partitioned by destination window (128 dst
  nodes), sorted by (window, src-half, dst-local).
- Per-edge source rows fetched with dma_gather (4 SWDGE queues round-robin,
  trailing -1 index padding so descriptor generation covers real edges only).
- Segment-sum via "staircase" matmul: S[e,n] = (pos_n >= e) built with one
  tensor_scalar is_ge per 128-edge tile; PSUM accumulates P[c,n] = prefix sums
  at the per-node boundary positions; adjacent-column differencing recovers
  per-node sums. Self-loops applied densely from the feature-major table.
- The two AllGather halves ARE the lo/hi gather tables (no copies); layer-2's
  first AG half fires mid-conv1.
- TimeEncode via scalar-engine Sin with range reduction (4 DVE + 1 ACT op).
- GRU / tab2 / BN-stats fused per conv window.
"""

import os

import numpy as np
import ml_dtypes

import concourse.bass as bass
import concourse.bacc as bacc
import concourse.mybir as mybir
import concourse.tile as tile
from concourse.bass_utils import run_bass_kernel_spmd

BF = ml_dtypes.bfloat16
F32 = mybir.dt.float32
BF16 = mybir.dt.bfloat16
I16 = mybir.dt.int16
I32 = mybir.dt.int32
AL = mybir.AluOpType
AF = mybir.ActivationFunctionType

N = 50000
NV = 50176
C = 128
NCORES = 8
NLOC = NV // NCORES     # 6272
NW = NLOC // 128        # 49
HALF_LOC = 3072         # 24 windows -> AG-a; 25 windows -> AG-b
NWA = HALF_LOC // 128   # 24
NLO = NCORES * HALF_LOC          # 24576 rows in lo table
NHI = NCORES * (NLOC - HALF_LOC)  # 25600 rows in hi table

LAST_EXEC_NS = None
LAST_RESULTS = None


# ---------------------------------------------------------------------------
# host preprocessing
# ---------------------------------------------------------------------------

def _hilo(a):
    a = np.asarray(a, np.float32)
    hi = a.astype(BF)
    lo = (a - hi.astype(np.float32)).astype(BF)
    return np.stack([hi, lo], 0)


def _preprocess(inp):
    src = np.asarray(inp["edge_index"][0], np.int64)
    dst = np.asarray(inp["edge_index"][1], np.int64)

    # degree includes the self-loop (handled densely on device)
    deg = np.bincount(dst, minlength=NV).astype(np.float32) + 1.0
    deg[N:] = 0.0
    dinv = np.where(deg > 0, 1.0 / np.sqrt(deg), 0.0).astype(np.float32)

    # src -> (half, row16) in the AG-produced half tables
    score = src // NLOC
    swithin = src % NLOC
    half = (swithin >= HALF_LOC).astype(np.int64)
    row16 = np.where(half == 0,
                     score * HALF_LOC + swithin,
                     score * (NLOC - HALF_LOC) + (swithin - HALF_LOC))

    dcore = dst // NLOC
    win = (dst % NLOC) // 128
    nl = dst % 128

    order = np.lexsort((nl, half, win, dcore))
    dcore_s = dcore[order]
    win_s = win[order]
    half_s = half[order]
    nl_s = nl[order]
    row_s = row16[order]

    # per (core, win, half) counts -> compile-time tile counts (max over cores)
    key = (dcore_s * NW + win_s) * 2 + half_s
    nkeys = NCORES * NW * 2
    cnt = np.bincount(key, minlength=nkeys).reshape(NCORES, NW, 2)
    tcnt = np.ceil(cnt.max(axis=0) / 128.0).astype(np.int64)  # [NW, 2]
    tcnt = np.maximum(tcnt, 1)
    TMAX = int(tcnt.max())
    off_slots = np.zeros((NW, 2), np.int64)  # slot offset (units of 128 idxs)
    acc = 0
    for w in range(NW):
        for h in range(2):
            off_slots[w, h] = acc
            acc += tcnt[w, h]
    TOTT = int(acc)

    starts = np.zeros(nkeys + 1, np.int64)
    np.cumsum(cnt.reshape(-1), out=starts[1:])
    pos_in_key = np.arange(len(key)) - starts[key]

    per_core = []
    for k in range(NCORES):
        sel = dcore_s == k
        w_ = win_s[sel]
        h_ = half_s[sel]
        p_ = pos_in_key[sel]
        r_ = row_s[sel]
        n_ = nl_s[sel]

        # NOTE: -1 trailing-pad (descriptor trim) hangs the runtime; pad 0.
        pad_val = -1 if os.environ.get("K_TRIM", "0") == "1" else 0
        idx_arr = np.full((16, TOTT * 8), pad_val, np.int16)
        j = off_slots[w_, h_] * 128 + p_      # global idx slot
        idx_arr[j % 16, (j // 16)] = r_.astype(np.int16)

        # pos[n] per (w, h): (# edges with nl <= n) - 1
        pos = np.full((NW, 2, 128), -1.0, np.float32)
        cnt_wh = np.zeros((NW, 2, 128), np.int64)
        np.add.at(cnt_wh, (w_, h_, n_), 1)
        pos[:, :, :] = np.cumsum(cnt_wh, axis=2) - 1.0
        pos_rep = np.broadcast_to(
            pos.reshape(1, NW * 2 * 128), (128, NW * 2 * 128)).copy()

        lo, hi_ = k * NLOC, (k + 1) * NLOC
        d = dict(
            idx_all=np.tile(idx_arr, (8, 1)),
            pos_all=pos_rep.astype(np.float32),
        )
        nrow = max(0, min(NLOC, N - lo))
        nfp = np.zeros((NLOC, C), np.float32)
        ts_p = np.zeros(NLOC, np.float32)
        xp1 = np.zeros((NLOC, C), np.float32)
        xp2 = np.zeros((NLOC, C), np.float32)
        if nrow > 0:
            nfp[:nrow] = np.asarray(inp["node_features"][lo:lo + nrow],
                                    np.float32)
            ts_p[:nrow] = np.asarray(inp["ts"][lo:lo + nrow],
                                     np.float32).reshape(-1)
            xp1[:nrow] = np.asarray(inp["x_prev1"][lo:lo + nrow], np.float32)
            xp2[:nrow] = np.asarray(inp["x_prev2"][lo:lo + nrow], np.float32)
        d["nf_fm"] = np.ascontiguousarray(nfp.T.astype(BF))
        d["ts_rep"] = np.broadcast_to(ts_p.reshape(1, NLOC),
                                      (128, NLOC)).copy()
        d["xp1_fm"] = np.ascontiguousarray(xp1.T)
        d["xp1_fmb"] = np.ascontiguousarray(xp1.T.astype(BF))
        d["xp2_fm"] = np.ascontiguousarray(xp2.T)
        d["xp2_fmb"] = np.ascontiguousarray(xp2.T.astype(BF))
        dv = dinv[lo:hi_]
        d["dinv_nm"] = np.ascontiguousarray(dv.reshape(NW, 128).T)
        d["dinvb"] = np.broadcast_to(dv.reshape(1, NLOC), (128, NLOC)).copy()
        mask = np.zeros((1, NLOC), np.float32)
        mask[0, :nrow] = 1.0
        d["mask_row"] = mask.astype(BF)
        per_core.append(d)

    # shared consts
    freq = np.asarray(inp["basis_freq"], np.float64)
    freq2_col = (freq / (2 * np.pi)).astype(np.float32).reshape(C, 1)
    # te_stored = -cos(ts*freq + phase) = sin(2pi*(y - 0.25)), y the turns.
    # u = y + 0.75 (>0, same angle mod 1); g = reduce(u) in [-0.5, 0.5]
    # robust to either int-conversion rounding mode; te = Sin(g * 2pi).
    phq_col = (np.asarray(inp["phase"], np.float64) / (2 * np.pi)
               + 0.75).astype(np.float32).reshape(C, 1)
    mpi_col = np.full((C, 1), -np.pi, np.float32)

    mW = np.asarray(inp["merge_W"], np.float64)
    W1_ = np.asarray(inp["W1"], np.float64)
    W2_ = np.asarray(inp["W2"], np.float64)
    sW = np.asarray(inp["skip_W"], np.float64)
    M1 = mW.T @ W1_.T
    S1 = mW.T @ sW.T
    b_m = np.asarray(inp["merge_b"], np.float64)

    consts = dict(
        R1a=M1[:C].astype(BF), R1b=(-M1[C:]).astype(BF),
        S1a=S1[:C].astype(BF), S1b=(-S1[C:]).astype(BF),
        W2T=W2_.T.astype(BF),
        tab1_bias2=_hilo(b_m @ W1_.T).reshape(2, C),
        skip_bias2=_hilo(b_m @ sW.T +
                         np.asarray(inp["skip_b"], np.float64)).reshape(2, C),
        b1_col=np.asarray(inp["b1"], np.float32).reshape(C, 1),
        b2_col=np.asarray(inp["b2"], np.float32).reshape(C, 1),
        freq2_col=freq2_col, phq_col=phq_col, mpi_col=mpi_col,
        iotaT=(np.arange(128, dtype=np.float32).reshape(128, 1)
               + 128.0 * np.arange(TMAX, dtype=np.float32).reshape(1, TMAX)),
        ident_f=np.eye(128, dtype=np.float32),
        ident_b=np.eye(128, dtype=np.float32).astype(BF),
    )
    for l in (1, 2):
        Wih = np.asarray(inp[f"gru{l}_Wih"], np.float32)
        Whh = np.asarray(inp[f"gru{l}_Whh"], np.float32)
        bih = np.asarray(inp[f"gru{l}_bih"], np.float32)
        bhh = np.asarray(inp[f"gru{l}_bhh"], np.float32)
        for gi, gate in enumerate("rzn"):
            consts[f"g{l}Wi{gate}"] = Wih[gi * C:(gi + 1) * C].T.astype(BF)
            consts[f"g{l}Wh{gate}"] = Whh[gi * C:(gi + 1) * C].T.astype(BF)
        consts[f"g{l}brz_r"] = (bih[0:C] + bhh[0:C]).reshape(C, 1)
        consts[f"g{l}brz_z"] = (bih[C:2 * C] + bhh[C:2 * C]).reshape(C, 1)
        consts[f"g{l}bin"] = bih[2 * C:].reshape(C, 1)
        consts[f"g{l}bhn"] = bhh[2 * C:].reshape(C, 1)

    for d in per_core:
        d.update(consts)

    static = dict(tcnt=tcnt.tolist(), off_slots=off_slots.tolist(),
                  TOTT=TOTT, TMAX=TMAX)
    return per_core, static


# ---------------------------------------------------------------------------
# bass program
# ---------------------------------------------------------------------------

def _build(nc, static):
    PH = int(os.environ.get("K_PH", "9"))
    NQ = int(os.environ.get("K_NQ", "4"))
    tcnt = static["tcnt"]
    off_slots = static["off_slots"]
    TOTT = static["TOTT"]
    TMAX = static["TMAX"]

    def din(name, shape, dt):
        return nc.dram_tensor(name, shape, dt, kind="ExternalInput")

    idx_all = din("idx_all", [128, TOTT * 8], I16)
    pos_all = din("pos_all", [128, NW * 2 * 128], F32)
    nf_fm = din("nf_fm", [128, NLOC], BF16)
    ts_rep = din("ts_rep", [128, NLOC], F32)
    xp1_fm = din("xp1_fm", [128, NLOC], F32)
    xp1_fmb = din("xp1_fmb", [128, NLOC], BF16)
    xp2_fm = din("xp2_fm", [128, NLOC], F32)
    xp2_fmb = din("xp2_fmb", [128, NLOC], BF16)
    dinv_nm = din("dinv_nm", [128, NW], F32)
    dinvb = din("dinvb", [128, NLOC], F32)
    mask_row = din("mask_row", [1, NLOC], BF16)

    cn = {}
    for nm, shape, dt in [
        ("R1a", [C, C], BF16), ("R1b", [C, C], BF16),
        ("S1a", [C, C], BF16), ("S1b", [C, C], BF16),
        ("W2T", [C, C], BF16),
        ("tab1_bias2", [2, C], BF16), ("skip_bias2", [2, C], BF16),
        ("b1_col", [C, 1], F32), ("b2_col", [C, 1], F32),
        ("freq2_col", [C, 1], F32), ("phq_col", [C, 1], F32),
        ("mpi_col", [C, 1], F32),
        ("iotaT", [128, TMAX], F32),
        ("ident_f", [128, 128], F32), ("ident_b", [128, 128], BF16),
    ]:
        cn[nm] = din(nm, shape, dt)
    for l in (1, 2):
        for gate in "rzn":
            cn[f"g{l}Wi{gate}"] = din(f"g{l}Wi{gate}", [C, C], BF16)
            cn[f"g{l}Wh{gate}"] = din(f"g{l}Wh{gate}", [C, C], BF16)
        for nm in ("brz_r", "brz_z", "bin", "bhn"):
            cn[f"g{l}{nm}"] = din(f"g{l}{nm}", [C, 1], F32)

    h1_out = nc.dram_tensor("h1_out", [NLOC, C], F32, kind="ExternalOutput")
    h2_out = nc.dram_tensor("h2_out", [NLOC, C], F32, kind="ExternalOutput")
    DBG = os.environ.get("K_DBG", "0") == "1"
    if DBG:
        te_dbg = nc.dram_tensor("te_dbg", [128, NLOC], BF16,
                                kind="ExternalOutput")
        hc_dbg = nc.dram_tensor("hc_dbg", [128, NLOC], F32,
                                kind="ExternalOutput")

    tab_loc_a = [nc.dram_tensor(f"tab{l}_loc_a", [HALF_LOC, C], BF16)
                 for l in (1, 2)]
    tab_loc_b = [nc.dram_tensor(f"tab{l}_loc_b", [NLOC - HALF_LOC, C], BF16)
                 for l in (1, 2)]
    tab_glo = [nc.dram_tensor(f"tab{l}_glo", [NLO, C], BF16,
                              addr_space="Shared") for l in (1, 2)]
    tab_ghi = [nc.dram_tensor(f"tab{l}_ghi", [NHI, C], BF16,
                              addr_space="Shared") for l in (1, 2)]
    COPYTAB = os.environ.get("K_COPYTAB", "0") == "1"
    AG2LATE = os.environ.get("K_AG2LATE", "0") == "1"
    if COPYTAB:
        tab_glo_l = [nc.dram_tensor(f"tab{l}_glo_l", [NLO, C], BF16)
                     for l in (1, 2)]
        tab_ghi_l = [nc.dram_tensor(f"tab{l}_ghi_l", [NHI, C], BF16)
                     for l in (1, 2)]
    bn_in = nc.dram_tensor("bn_in", [128, 2], F32)
    bn_out = nc.dram_tensor("bn_out", [128, 2], F32, addr_space="Shared")

    RG = [list(range(NCORES))]

    with tile.TileContext(nc) as tc:
        res_cm = tc.tile_pool(name="res", bufs=1)
        res = res_cm.__enter__()

        # ---- resident tiles ----
        nf_t = res.tile([128, NLOC], BF16, name="nf_t")
        nc.sync.dma_start(nf_t[:], nf_fm[:])
        te_t = res.tile([128, NLOC], BF16, name="te_t")
        dinvb_t = res.tile([128, NLOC], F32, name="dinvb_t")
        nc.sync.dma_start(dinvb_t[:], dinvb[:])
        dinv_nm_t = res.tile([128, NW], F32, name="dinv_nm_t")
        nc.sync.dma_start(dinv_nm_t[:], dinv_nm[:])
        tabfm = [res.tile([128, NLOC], BF16, name=f"tabfm{l}") for l in (1, 2)]
        Hpre_t = res.tile([128, NLOC], F32, name="Hpre_t")
        mask_t = res.tile([1, NLOC], BF16, name="mask_t")
        nc.sync.dma_start(mask_t[:], mask_row[:])

        w_t = {}
        for nm in cn:
            shape = list(cn[nm].shape)
            w_t[nm] = res.tile(shape, cn[nm].dtype, name=f"w_{nm}")
            nc.sync.dma_start(w_t[nm][:], cn[nm][:])
        ones2 = res.tile([2, 512], BF16, name="ones2")
        nc.vector.memset(ones2[:], 1.0)
        zero_col = res.tile([128, 1], F32, name="zero_col")
        nc.vector.memset(zero_col[:], 0.0)
        part_s = res.tile([128, NW], F32, name="part_s")
        part_q = res.tile([128, NW], F32, name="part_q")
        msum = res.tile([128, 2], F32, name="msum")
        bnred = res.tile([128, 2], F32, name="bnred")
        mean_c = res.tile([128, 1], F32, name="mean_c")
        istd_c = res.tile([128, 1], F32, name="istd_c")

        # ================= phase 1: t_embed via Sin =================
        with tc.tile_pool(name="p1", bufs=1) as p1:
            CH = 1568
            for lo in range(0, NLOC, CH):
                cs = slice(lo, lo + CH)
                tsr = p1.tile([128, CH], F32, name="tsr", tag="tsr", bufs=2)
                nc.sync.dma_start(tsr[:], ts_rep[:, cs])
                u = p1.tile([128, CH], F32, name="u", tag="u", bufs=1)
                nc.vector.tensor_scalar(u[:], tsr[:],
                                        w_t["freq2_col"][:],
                                        w_t["phq_col"][:],
                                        op0=AL.mult, op1=AL.add)
                ui = p1.tile([128, CH], I32, name="ui", tag="ui", bufs=1)
                nc.vector.tensor_copy(ui[:], u[:])
                uf = p1.tile([128, CH], F32, name="uf", tag="uf", bufs=1)
                nc.vector.tensor_copy(uf[:], ui[:])
                f = p1.tile([128, CH], F32, name="f", tag="f", bufs=1)
                nc.vector.tensor_tensor(f[:], u[:], uf[:], op=AL.subtract)
                st = p1.tile([128, CH], F32, name="st", tag="st", bufs=1)
                nc.vector.tensor_scalar(st[:], f[:], 0.5, None, op0=AL.is_ge)
                g = p1.tile([128, CH], F32, name="g", tag="g", bufs=1)
                nc.vector.tensor_tensor(g[:], f[:], st[:], op=AL.subtract)
                nc.scalar.activation(te_t[:, cs], g[:], AF.Sin,
                                     bias=0.0, scale=float(2 * np.pi))
                if DBG:
                    nc.sync.dma_start(te_dbg[:, cs], te_t[:, cs])

        # ================= tab production helper =================
        def tab_prod(l, w, produce, tp, tps):
            """produce(pt, ws): node-major [n, c] psum for window w."""
            ws = slice(w * 128, (w + 1) * 128)
            pt = tps.tile([128, 128], F32, name="pt", tag="pt", bufs=1)
            produce(pt, ws)
            ot = tp.tile([128, 128], BF16, name="ot", tag="ot", bufs=3)
            nc.vector.tensor_scalar(ot[:], pt[:], dinv_nm_t[:, w:w + 1],
                                    None, op0=AL.mult)
            if w < NWA:
                nc.sync.dma_start(tab_loc_a[l - 1][ws, :], ot[:])
            else:
                ws2 = slice((w - NWA) * 128, (w - NWA + 1) * 128)
                nc.sync.dma_start(tab_loc_b[l - 1][ws2, :], ot[:])
            ptf = tps.tile([128, 128], BF16, name="ptf", tag="ptf", bufs=1)
            nc.tensor.transpose(ptf[:], ot[:], w_t["ident_b"][:])
            nc.vector.tensor_copy(tabfm[l - 1][:, ws], ptf[:])

        def fire_ag(l, part):
            if part == 0:
                nc.gpsimd.collective_compute(
                    "AllGather", AL.bypass, replica_groups=RG,
                    ins=[tab_loc_a[l - 1][:]], outs=[tab_glo[l - 1][:]])
                if COPYTAB:
                    nc.sync.dma_start(tab_glo_l[l - 1][:], tab_glo[l - 1][:])
            else:
                nc.gpsimd.collective_compute(
                    "AllGather", AL.bypass, replica_groups=RG,
                    ins=[tab_loc_b[l - 1][:]], outs=[tab_ghi[l - 1][:]])
                if COPYTAB:
                    nc.sync.dma_start(tab_ghi_l[l - 1][:], tab_ghi[l - 1][:])

        # ---- tab1 ----
        def prod1(pt, ws):
            nc.tensor.matmul(pt[:], nf_t[:, ws], w_t["R1a"][:],
                             start=True, stop=False)
            nc.tensor.matmul(pt[:], te_t[:, ws], w_t["R1b"][:],
                             start=False, stop=False)
            nc.tensor.matmul(pt[:], ones2[:, 0:128], w_t["tab1_bias2"][:],
                             start=False, stop=True)

        with tc.tile_pool(name="tab1", bufs=1) as tp, \
             tc.tile_pool(name="tab1ps", bufs=1, space="PSUM") as tps:
            for w in range(NW):
                tab_prod(1, w, prod1, tp, tps)
                if w == NWA - 1:
                    fire_ag(1, 0)
            fire_ag(1, 1)

        # ================= GRU window helper =================
        def gru_win(l, Hcb, xf, xfb, gp, gps):
            def mm2(wi, wh, tag):
                pi = gps.tile([128, 128], F32, name=tag, tag="pi", bufs=2)
                nc.tensor.matmul(pi[:], w_t[wi][:], Hcb[:],
                                 start=True, stop=False)
                nc.tensor.matmul(pi[:], w_t[wh][:], xfb[:],
                                 start=False, stop=True)
                return pi

            smr = mm2(f"g{l}Wir", f"g{l}Whr", "smr")
            r = gp.tile([128, 128], F32, name="r", tag="r", bufs=2)
            nc.scalar.activation(r[:], smr[:], AF.Sigmoid,
                                 bias=w_t[f"g{l}brz_r"][:])
            smz = mm2(f"g{l}Wiz", f"g{l}Whz", "smz")
            z = gp.tile([128, 128], F32, name="z", tag="z", bufs=2)
            nc.scalar.activation(z[:], smz[:], AF.Sigmoid,
                                 bias=w_t[f"g{l}brz_z"][:])
            pin = gps.tile([128, 128], F32, name="pin", tag="pi", bufs=2)
            nc.tensor.matmul(pin[:], w_t[f"g{l}Win"][:], Hcb[:],
                             start=True, stop=True)
            phn = gps.tile([128, 128], F32, name="phn", tag="pi", bufs=2)
            nc.tensor.matmul(phn[:], w_t[f"g{l}Whn"][:], xfb[:],
                             start=True, stop=True)
            hn = gp.tile([128, 128], F32, name="hn", tag="hn", bufs=2)
            nc.vector.tensor_scalar(hn[:], phn[:], w_t[f"g{l}bhn"][:],
                                    None, op0=AL.add)
            rn = gp.tile([128, 128], F32, name="rn", tag="rn", bufs=2)
            nc.vector.tensor_tensor(rn[:], r[:], hn[:], op=AL.mult)
            t2 = gp.tile([128, 128], F32, name="t2", tag="t2", bufs=2)
            nc.vector.tensor_tensor(t2[:], pin[:], rn[:], op=AL.add)
            ng = gp.tile([128, 128], F32, name="ng", tag="ng", bufs=2)
            nc.scalar.activation(ng[:], t2[:], AF.Tanh,
                                 bias=w_t[f"g{l}bin"][:])
            d = gp.tile([128, 128], F32, name="d", tag="d", bufs=2)
            nc.vector.tensor_tensor(d[:], xf[:], ng[:], op=AL.subtract)
            zd = gp.tile([128, 128], F32, name="zd", tag="zd", bufs=2)
            nc.vector.tensor_tensor(zd[:], z[:], d[:], op=AL.mult)
            H = gp.tile([128, 128], F32, name="H", tag="H", bufs=2)
            nc.vector.tensor_tensor(H[:], ng[:], zd[:], op=AL.add)
            return H

        # ================= conv layer =================
        def conv_layer(l, b_col, xf_dram, xfb_dram, fin):
            tglo = tab_glo_l[l - 1] if COPYTAB else tab_glo[l - 1]
            tghi = tab_ghi_l[l - 1] if COPYTAB else tab_ghi[l - 1]
            with tc.tile_pool(name=f"cv{l}", bufs=1) as gp, \
                 tc.tile_pool(name=f"cv{l}ps", bufs=1, space="PSUM") as cps:
                for w in range(NW):
                    ws = slice(w * 128, (w + 1) * 128)
                    t_lo, t_hi = tcnt[w][0], tcnt[w][1]
                    base8 = [off_slots[w][0] * 8, off_slots[w][1] * 8]

                    it_lo = gp.tile([128, TMAX * 8], I16, name="it_lo",
                                    tag="it_lo", bufs=4)
                    nc.sync.dma_start(
                        it_lo[:, :t_lo * 8],
                        idx_all[:, base8[0]:base8[0] + t_lo * 8])
                    it_hi = gp.tile([128, TMAX * 8], I16, name="it_hi",
                                    tag="it_hi", bufs=4)
                    nc.sync.dma_start(
                        it_hi[:, :t_hi * 8],
                        idx_all[:, base8[1]:base8[1] + t_hi * 8])
                    pos_lo = gp.tile([128, 128], F32, name="pos_lo",
                                     tag="pos_lo", bufs=4)
                    nc.sync.dma_start(
                        pos_lo[:], pos_all[:, (w * 2) * 128:(w * 2 + 1) * 128])
                    pos_hi = gp.tile([128, 128], F32, name="pos_hi",
                                     tag="pos_hi", bufs=4)
                    nc.sync.dma_start(
                        pos_hi[:],
                        pos_all[:, (w * 2 + 1) * 128:(w * 2 + 2) * 128])

                    glo = gp.tile([128, TMAX, 128], BF16, name="glo",
                                  tag="glo", bufs=3)
                    ghi = gp.tile([128, TMAX, 128], BF16, name="ghi",
                                  tag="ghi", bufs=3)
                    if w < 3:
                        nc.vector.memset(glo[:], 0.0)
                        nc.vector.memset(ghi[:], 0.0)
                    nc.gpsimd.dma_gather(
                        glo[:, :t_lo, :], tglo[:], it_lo[:, :t_lo * 8],
                        t_lo * 128, t_lo * 128, 128,
                        single_packet=False, queue_num=(2 * w) % NQ)
                    nc.gpsimd.dma_gather(
                        ghi[:, :t_hi, :], tghi[:], it_hi[:, :t_hi * 8],
                        t_hi * 128, t_hi * 128, 128,
                        single_packet=False, queue_num=(2 * w + 1) % NQ)

                    ps = cps.tile([128, 128], F32, name="ps", tag="ps",
                                  bufs=2)
                    for t in range(t_lo):
                        S = gp.tile([128, 128], BF16, name="S", tag="S",
                                    bufs=6)
                        nc.vector.tensor_scalar(
                            S[:], pos_lo[:], w_t["iotaT"][:, t:t + 1],
                            None, op0=AL.is_ge)
                        nc.tensor.matmul(ps[:], glo[:, t, :], S[:],
                                         start=(t == 0), stop=False)
                    for t in range(t_hi):
                        S = gp.tile([128, 128], BF16, name="S", tag="S",
                                    bufs=6)
                        nc.vector.tensor_scalar(
                            S[:], pos_hi[:], w_t["iotaT"][:, t:t + 1],
                            None, op0=AL.is_ge)
                        nc.tensor.matmul(ps[:], ghi[:, t, :], S[:],
                                         start=False, stop=(t == t_hi - 1))

                    # epilogue: diff -> +selfloop -> *dinv -> +b
                    pcp = gp.tile([128, 128], F32, name="pcp", tag="pcp",
                                  bufs=2)
                    nc.vector.tensor_copy(pcp[:], ps[:])
                    d0 = gp.tile([128, 128], F32, name="d0", tag="d0", bufs=2)
                    nc.vector.tensor_copy(d0[:, 0:1], pcp[:, 0:1])
                    nc.vector.tensor_tensor(d0[:, 1:128], pcp[:, 1:128],
                                            pcp[:, 0:127], op=AL.subtract)
                    d1 = gp.tile([128, 128], F32, name="d1", tag="d1", bufs=2)
                    nc.vector.tensor_tensor(d1[:], d0[:],
                                            tabfm[l - 1][:, ws], op=AL.add)
                    d2 = gp.tile([128, 128], F32, name="d2", tag="d2", bufs=2)
                    nc.vector.tensor_tensor(d2[:], d1[:], dinvb_t[:, ws],
                                            op=AL.mult)
                    Hc = gp.tile([128, 128], F32, name="Hc", tag="Hc", bufs=2)
                    nc.vector.tensor_scalar(Hc[:], d2[:], b_col, None,
                                            op0=AL.add)
                    Hcb = gp.tile([128, 128], BF16, name="Hcb", tag="Hcb",
                                  bufs=2)
                    nc.vector.tensor_copy(Hcb[:], Hc[:])
                    if DBG and l == 1:
                        nc.sync.dma_start(hc_dbg[:, ws], Hc[:])

                    xf = gp.tile([128, 128], F32, name="xf", tag="xf", bufs=3)
                    nc.sync.dma_start(xf[:], xf_dram[:, ws])
                    xfb = gp.tile([128, 128], BF16, name="xfb", tag="xfb",
                                  bufs=3)
                    nc.sync.dma_start(xfb[:], xfb_dram[:, ws])

                    H = gru_win(l, Hcb, xf, xfb, gp, cps)
                    fin(w, ws, H, gp, cps)

        # ---- layer 1 ----
        def fin1(w, ws, H, gp, gps):
            Hr = gp.tile([128, 128], F32, name="Hr", tag="Hr", bufs=2)
            nc.scalar.activation(Hr[:], H[:], AF.Relu, bias=zero_col[:])
            H1b = gp.tile([128, 128], BF16, name="H1b", tag="H1b", bufs=2)
            nc.vector.tensor_copy(H1b[:], Hr[:])
            # h1 out (transpose to node-major)
            ptr = gps.tile([128, 128], F32, name="ptr", tag="ptr", bufs=1)
            nc.tensor.transpose(ptr[:], Hr[:], w_t["ident_f"][:])
            ob = gp.tile([128, 128], F32, name="ob", tag="ob", bufs=3)
            nc.scalar.copy(ob[:], ptr[:])
            nc.sync.dma_start(h1_out[ws, :], ob[:])

            # tab2 production
            def prod2(pt, ws_):
                nc.tensor.matmul(pt[:], H1b[:], w_t["W2T"][:],
                                 start=True, stop=True)
            tab_prod(2, w, prod2, gp, gps)
            if not AG2LATE:
                if w == NWA - 1:
                    fire_ag(2, 0)
                if w == NW - 1:
                    fire_ag(2, 1)

        if PH >= 2:
            conv_layer(1, w_t["b1_col"][:], xp1_fm, xp1_fmb, fin1)
            if AG2LATE:
                fire_ag(2, 0)
                fire_ag(2, 1)

        # ---- layer 2 ----
        def fin2(w, ws, H, gp, gps):
            pk = gps.tile([128, 128], F32, name="pk", tag="pk", bufs=1)
            nc.tensor.matmul(pk[:], w_t["S1a"][:], nf_t[:, ws],
                             start=True, stop=False)
            nc.tensor.matmul(pk[:], w_t["S1b"][:], te_t[:, ws],
                             start=False, stop=False)
            nc.tensor.matmul(pk[:], w_t["skip_bias2"][:], ones2[:, 0:128],
                             start=False, stop=True)
            nc.vector.tensor_tensor(Hpre_t[:, ws], H[:], pk[:], op=AL.add)
            # BN partial stats
            pm = gps.tile([128, 128], F32, name="pm", tag="pm", bufs=1)
            nc.tensor.matmul(pm[:], ones2[0:1, 0:128], mask_t[:, ws],
                             start=True, stop=True)
            hm = gp.tile([128, 128], F32, name="hm", tag="hm", bufs=2)
            nc.vector.tensor_tensor(hm[:], Hpre_t[:, ws], pm[:], op=AL.mult)
            nc.vector.tensor_reduce(part_s[:, w:w + 1], hm[:],
                                    axis=mybir.AxisListType.X, op=AL.add)
            sqs = gp.tile([128, 128], F32, name="sqs", tag="sqs", bufs=2)
            nc.scalar.activation(sqs[:], hm[:], AF.Square, bias=0.0,
                                 accum_out=part_q[:, w:w + 1])

        if PH >= 3:
            conv_layer(2, w_t["b2_col"][:], xp2_fm, xp2_fmb, fin2)
        else:
            nc.vector.memset(Hpre_t[:], 0.0)
            nc.vector.memset(part_s[:], 0.0)
            nc.vector.memset(part_q[:], 0.0)
            z1 = res.tile([128, 128], F32, name="z1")
            nc.vector.memset(z1[:], 0.0)
            if PH < 2:
                for w in range(NW):
                    nc.sync.dma_start(h1_out[w * 128:(w + 1) * 128, :], z1[:])

        # ================= BatchNorm finale =================
        with tc.tile_pool(name="bn", bufs=1) as bp, \
             tc.tile_pool(name="bnps", bufs=1, space="PSUM") as bps:
            nc.vector.tensor_reduce(msum[:, 0:1], part_s[:],
                                    axis=mybir.AxisListType.X, op=AL.add)
            nc.vector.tensor_reduce(msum[:, 1:2], part_q[:],
                                    axis=mybir.AxisListType.X, op=AL.add)
            nc.sync.dma_start(bn_in[:], msum[:])
            nc.gpsimd.collective_compute(
                "AllReduce", AL.add, replica_groups=RG,
                ins=[bn_in[:]], outs=[bn_out[:]])
            nc.sync.dma_start(bnred[:], bn_out[:])
            nc.vector.tensor_scalar(mean_c[:], bnred[:, 0:1], 1.0 / N, None,
                                    op0=AL.mult)
            m2 = bp.tile([128, 1], F32, name="m2")
            nc.vector.tensor_tensor(m2[:], mean_c[:], mean_c[:], op=AL.mult)
            v1 = bp.tile([128, 1], F32, name="v1")
            nc.vector.tensor_scalar(v1[:], bnred[:, 1:2], 1.0 / N, None,
                                    op0=AL.mult)
            v2 = bp.tile([128, 1], F32, name="v2")
            nc.vector.tensor_tensor(v2[:], v1[:], m2[:], op=AL.subtract)
            v3 = bp.tile([128, 1], F32, name="v3")
            nc.vector.tensor_scalar(v3[:], v2[:], 1e-5, None, op0=AL.add)
            v4 = bp.tile([128, 1], F32, name="v4")
            nc.scalar.activation(v4[:], v3[:], AF.Sqrt, bias=zero_col[:])
            nc.vector.reciprocal(istd_c[:], v4[:])
            for w in range(NW):
                ws = slice(w * 128, (w + 1) * 128)
                hn_ = bp.tile([128, 128], F32, name="hn_", tag="hn_", bufs=2)
                nc.vector.tensor_scalar(hn_[:], Hpre_t[:, ws],
                                        mean_c[:], istd_c[:],
                                        op0=AL.subtract, op1=AL.mult)
                ptr = bps.tile([128, 128], F32, name="ptr", tag="ptr", bufs=2)
                nc.tensor.transpose(ptr[:], hn_[:], w_t["ident_f"][:])
                ob = bp.tile([128, 128], F32, name="ob", tag="ob", bufs=3)
                nc.scalar.copy(ob[:], ptr[:])
                nc.sync.dma_start(h2_out[ws, :], ob[:])

        res_cm.__exit__(None, None, None)
    return nc


# ---------------------------------------------------------------------------
# entry point
# ---------------------------------------------------------------------------

def _install_ntff_hook():
    """Install antenv.axon_hooks (missing in this image) for trace=True."""
    import sys
    import types
    try:
        import antenv
        if getattr(antenv, "axon_hooks", None) is not None:
            return
        from trn_agent_boot.trn_boot import _ntff_profile_via_ctypes
        hook = _ntff_profile_via_ctypes("/opt/axon/libaxon_pjrt.so")
        mod = types.ModuleType("antenv.axon_hooks")
        mod.set_axon_ntff_profile_hook = lambda h: None
        mod.get_axon_ntff_profile_hook = lambda: hook
        sys.modules["antenv.axon_hooks"] = mod
        antenv.axon_hooks = mod
    except Exception:
        pass


def kernel(**inputs):
    global LAST_EXEC_NS
    per_core, static = _preprocess(inputs)

    nc = bacc.Bacc("TRN2", target_bir_lowering=False, debug=False,
                   num_devices=NCORES, num_swdge_queues=4)
    _build(nc, static)
    nc.compile()

    in_maps = [per_core[k] for k in range(NCORES)]
    trace = os.environ.get("KERNEL_TRACE", "0") == "1"
    if trace:
        _install_ntff_hook()
    res = run_bass_kernel_spmd(nc, in_maps, list(range(NCORES)), trace=trace)
    LAST_EXEC_NS = res.exec_time_ns
    global LAST_RESULTS
    LAST_RESULTS = res.results

    H1 = np.zeros((N, C), np.float32)
    H2 = np.zeros((N, C), np.float32)
    for k in range(NCORES):
        lo, hi_ = k * NLOC, min((k + 1) * NLOC, N)
        if lo >= N:
            break
        nrow = hi_ - lo
        H1[lo:hi_] = res.results[k]["h1_out"][:nrow]
        H2[lo:hi_] = res.results[k]["h2_out"][:nrow]
    return (H1, H2)


# revision 13
# speedup vs baseline: 1.0339x; 1.0339x over previous
"""Trainium2 Bass kernel for nn_DynamicGCNModel (2-layer GCN+GRU, 50k nodes,
1.6M edges, C=128) on 8 NeuronCores.

v2 design:
- Nodes sharded 6272/core; edges partitioned by destination window (128 dst
  nodes), sorted by (window, src-half, dst-local).
- Per-edge source rows fetched with dma_gather (4 SWDGE queues round-robin,
  trailing -1 index padding so descriptor generation covers real edges only).
- Segment-sum via "staircase" matmul: S[e,n] = (pos_n >= e) built with one
  tensor_scalar is_ge per 128-edge tile; PSUM accumulates P[c,n] = prefix sums
  at the per-node boundary positions; adjacent-column differencing recovers
  per-node sums. Self-loops applied densely from the feature-major table.
- The two AllGather halves ARE the lo/hi gather tables (no copies); layer-2's
  first AG half fires mid-conv1.
- TimeEncode via scalar-engine Sin with range reduction (4 DVE + 1 ACT op).
- GRU / tab2 / BN-stats fused per conv window.
"""

import os

import numpy as np
import ml_dtypes

import concourse.bass as bass
import concourse.bacc as bacc
import concourse.mybir as mybir
import concourse.tile as tile
from concourse.bass_utils import run_bass_kernel_spmd

BF = ml_dtypes.bfloat16
F32 = mybir.dt.float32
BF16 = mybir.dt.bfloat16
I16 = mybir.dt.int16
I32 = mybir.dt.int32
AL = mybir.AluOpType
AF = mybir.ActivationFunctionType

N = 50000
NV = 50176
C = 128
NCORES = 8
NLOC = NV // NCORES     # 6272
NW = NLOC // 128        # 49
HALF_LOC = 3072         # 24 windows -> AG-a; 25 windows -> AG-b
NWA = HALF_LOC // 128   # 24
NLO = NCORES * HALF_LOC          # 24576 rows in lo table
NHI = NCORES * (NLOC - HALF_LOC)  # 25600 rows in hi table

LAST_EXEC_NS = None
LAST_RESULTS = None


# ---------------------------------------------------------------------------
# host preprocessing
# ---------------------------------------------------------------------------

def _hilo(a):
    a = np.asarray(a, np.float32)
    hi = a.astype(BF)
    lo = (a - hi.astype(np.float32)).astype(BF)
    return np.stack([hi, lo], 0)


def _preprocess(inp):
    src = np.asarray(inp["edge_index"][0], np.int64)
    dst = np.asarray(inp["edge_index"][1], np.int64)

    # degree includes the self-loop (handled densely on device)
    deg = np.bincount(dst, minlength=NV).astype(np.float32) + 1.0
    deg[N:] = 0.0
    dinv = np.where(deg > 0, 1.0 / np.sqrt(deg), 0.0).astype(np.float32)

    # src -> (half, row16) in the AG-produced half tables
    score = src // NLOC
    swithin = src % NLOC
    half = (swithin >= HALF_LOC).astype(np.int64)
    row16 = np.where(half == 0,
                     score * HALF_LOC + swithin,
                     score * (NLOC - HALF_LOC) + (swithin - HALF_LOC))

    dcore = dst // NLOC
    win = (dst % NLOC) // 128
    nl = dst % 128

    order = np.lexsort((nl, half, win, dcore))
    dcore_s = dcore[order]
    win_s = win[order]
    half_s = half[order]
    nl_s = nl[order]
    row_s = row16[order]

    # per (core, win, half) counts -> compile-time tile counts (max over cores)
    key = (dcore_s * NW + win_s) * 2 + half_s
    nkeys = NCORES * NW * 2
    cnt = np.bincount(key, minlength=nkeys).reshape(NCORES, NW, 2)
    tcnt = np.ceil(cnt.max(axis=0) / 128.0).astype(np.int64)  # [NW, 2]
    tcnt = np.maximum(tcnt, 1)
    TMAX = int(tcnt.max())
    off_slots = np.zeros((NW, 2), np.int64)  # slot offset (units of 128 idxs)
    acc = 0
    for w in range(NW):
        for h in range(2):
            off_slots[w, h] = acc
            acc += tcnt[w, h]
    TOTT = int(acc)

    starts = np.zeros(nkeys + 1, np.int64)
    np.cumsum(cnt.reshape(-1), out=starts[1:])
    pos_in_key = np.arange(len(key)) - starts[key]

    per_core = []
    for k in range(NCORES):
        sel = dcore_s == k
        w_ = win_s[sel]
        h_ = half_s[sel]
        p_ = pos_in_key[sel]
        r_ = row_s[sel]
        n_ = nl_s[sel]

        # NOTE: -1 trailing-pad (descriptor trim) hangs the runtime; pad 0.
        pad_val = -1 if os.environ.get("K_TRIM", "0") == "1" else 0
        idx_arr = np.full((16, TOTT * 8), pad_val, np.int16)
        j = off_slots[w_, h_] * 128 + p_      # global idx slot
        idx_arr[j % 16, (j // 16)] = r_.astype(np.int16)

        # pos[n] per (w, h): (# edges with nl <= n) - 1
        pos = np.full((NW, 2, 128), -1.0, np.float32)
        cnt_wh = np.zeros((NW, 2, 128), np.int64)
        np.add.at(cnt_wh, (w_, h_, n_), 1)
        pos[:, :, :] = np.cumsum(cnt_wh, axis=2) - 1.0
        pos_rep = np.broadcast_to(
            pos.reshape(1, NW * 2 * 128), (128, NW * 2 * 128)).copy()

        lo, hi_ = k * NLOC, (k + 1) * NLOC
        d = dict(
            idx_all=np.tile(idx_arr, (8, 1)),
            pos_all=pos_rep.astype(np.float32),
        )
        nrow = max(0, min(NLOC, N - lo))
        nfp = np.zeros((NLOC, C), np.float32)
        ts_p = np.zeros(NLOC, np.float32)
        xp1 = np.zeros((NLOC, C), np.float32)
        xp2 = np.zeros((NLOC, C), np.float32)
        if nrow > 0:
            nfp[:nrow] = np.asarray(inp["node_features"][lo:lo + nrow],
                                    np.float32)
            ts_p[:nrow] = np.asarray(inp["ts"][lo:lo + nrow],
                                     np.float32).reshape(-1)
            xp1[:nrow] = np.asarray(inp["x_prev1"][lo:lo + nrow], np.float32)
            xp2[:nrow] = np.asarray(inp["x_prev2"][lo:lo + nrow], np.float32)
        d["nf_fm"] = np.ascontiguousarray(nfp.T.astype(BF))
        d["ts_rep"] = np.broadcast_to(ts_p.reshape(1, NLOC),
                                      (128, NLOC)).copy()
        d["xp1_fm"] = np.ascontiguousarray(xp1.T)
        d["xp1_fmb"] = np.ascontiguousarray(xp1.T.astype(BF))
        d["xp2_fm"] = np.ascontiguousarray(xp2.T)
        d["xp2_fmb"] = np.ascontiguousarray(xp2.T.astype(BF))
        dv = dinv[lo:hi_]
        d["dinv_nm"] = np.ascontiguousarray(dv.reshape(NW, 128).T)
        d["dinvb"] = np.broadcast_to(dv.reshape(1, NLOC), (128, NLOC)).copy()
        mask = np.zeros((1, NLOC), np.float32)
        mask[0, :nrow] = 1.0
        d["mask_row"] = mask.astype(BF)
        per_core.append(d)

    # shared consts
    freq = np.asarray(inp["basis_freq"], np.float64)
    freq2_col = (freq / (2 * np.pi)).astype(np.float32).reshape(C, 1)
    # te_stored = -cos(ts*freq + phase) = sin(2pi*(y - 0.25)), y the turns.
    # u = y + 0.75 (>0, same angle mod 1); g = reduce(u) in [-0.5, 0.5]
    # robust to either int-conversion rounding mode; te = Sin(g * 2pi).
    phq_col = (np.asarray(inp["phase"], np.float64) / (2 * np.pi)
               + 0.75).astype(np.float32).reshape(C, 1)
    mpi_col = np.full((C, 1), -np.pi, np.float32)

    mW = np.asarray(inp["merge_W"], np.float64)
    W1_ = np.asarray(inp["W1"], np.float64)
    W2_ = np.asarray(inp["W2"], np.float64)
    sW = np.asarray(inp["skip_W"], np.float64)
    M1 = mW.T @ W1_.T
    S1 = mW.T @ sW.T
    b_m = np.asarray(inp["merge_b"], np.float64)

    consts = dict(
        R1a=M1[:C].astype(BF), R1b=(-M1[C:]).astype(BF),
        S1a=S1[:C].astype(BF), S1b=(-S1[C:]).astype(BF),
        W2T=W2_.T.astype(BF),
        tab1_bias2=_hilo(b_m @ W1_.T).reshape(2, C),
        skip_bias2=_hilo(b_m @ sW.T +
                         np.asarray(inp["skip_b"], np.float64)).reshape(2, C),
        b1_col=np.asarray(inp["b1"], np.float32).reshape(C, 1),
        b2_col=np.asarray(inp["b2"], np.float32).reshape(C, 1),
        freq2_col=freq2_col, phq_col=phq_col, mpi_col=mpi_col,
        iotaT=(np.arange(128, dtype=np.float32).reshape(128, 1)
               + 128.0 * np.arange(TMAX, dtype=np.float32).reshape(1, TMAX)),
        ident_f=np.eye(128, dtype=np.float32),
        ident_b=np.eye(128, dtype=np.float32).astype(BF),
    )
    for l in (1, 2):
        Wih = np.asarray(inp[f"gru{l}_Wih"], np.float32)
        Whh = np.asarray(inp[f"gru{l}_Whh"], np.float32)
        bih = np.asarray(inp[f"gru{l}_bih"], np.float32)
        bhh = np.asarray(inp[f"gru{l}_bhh"], np.float32)
        for gi, gate in enumerate("rzn"):
            consts[f"g{l}Wi{gate}"] = Wih[gi * C:(gi + 1) * C].T.astype(BF)
            consts[f"g{l}Wh{gate}"] = Whh[gi * C:(gi + 1) * C].T.astype(BF)
        consts[f"g{l}brz_r"] = (bih[0:C] + bhh[0:C]).reshape(C, 1)
        consts[f"g{l}brz_z"] = (bih[C:2 * C] + bhh[C:2 * C]).reshape(C, 1)
        consts[f"g{l}bin"] = bih[2 * C:].reshape(C, 1)
        consts[f"g{l}bhn"] = bhh[2 * C:].reshape(C, 1)

    for d in per_core:
        d.update(consts)

    static = dict(tcnt=tcnt.tolist(), off_slots=off_slots.tolist(),
                  TOTT=TOTT, TMAX=TMAX)
    return per_core, static


# ---------------------------------------------------------------------------
# bass program
# ---------------------------------------------------------------------------

def _build(nc, static):
    PH = int(os.environ.get("K_PH", "9"))
    NQ = int(os.environ.get("K_NQ", "4"))
    tcnt = static["tcnt"]
    off_slots = static["off_slots"]
    TOTT = static["TOTT"]
    TMAX = static["TMAX"]

    def din(name, shape, dt):
        return nc.dram_tensor(name, shape, dt, kind="ExternalInput")

    idx_all = din("idx_all", [128, TOTT * 8], I16)
    pos_all = din("pos_all", [128, NW * 2 * 128], F32)
    nf_fm = din("nf_fm", [128, NLOC], BF16)
    ts_rep = din("ts_rep", [128, NLOC], F32)
    xp1_fm = din("xp1_fm", [128, NLOC], F32)
    xp1_fmb = din("xp1_fmb", [128, NLOC], BF16)
    xp2_fm = din("xp2_fm", [128, NLOC], F32)
    xp2_fmb = din("xp2_fmb", [128, NLOC], BF16)
    dinv_nm = din("dinv_nm", [128, NW], F32)
    dinvb = din("dinvb", [128, NLOC], F32)
    mask_row = din("mask_row", [1, NLOC], BF16)

    cn = {}
    for nm, shape, dt in [
        ("R1a", [C, C], BF16), ("R1b", [C, C], BF16),
        ("S1a", [C, C], BF16), ("S1b", [C, C], BF16),
        ("W2T", [C, C], BF16),
        ("tab1_bias2", [2, C], BF16), ("skip_bias2", [2, C], BF16),
        ("b1_col", [C, 1], F32), ("b2_col", [C, 1], F32),
        ("freq2_col", [C, 1], F32), ("phq_col", [C, 1], F32),
        ("mpi_col", [C, 1], F32),
        ("iotaT", [128, TMAX], F32),
        ("ident_f", [128, 128], F32), ("ident_b", [128, 128], BF16),
    ]:
        cn[nm] = din(nm, shape, dt)
    for l in (1, 2):
        for gate in "rzn":
            cn[f"g{l}Wi{gate}"] = din(f"g{l}Wi{gate}", [C, C], BF16)
            cn[f"g{l}Wh{gate}"] = din(f"g{l}Wh{gate}", [C, C], BF16)
        for nm in ("brz_r", "brz_z", "bin", "bhn"):
            cn[f"g{l}{nm}"] = din(f"g{l}{nm}", [C, 1], F32)

    h1_out = nc.dram_tensor("h1_out", [NLOC, C], F32, kind="ExternalOutput")
    h2_out = nc.dram_tensor("h2_out", [NLOC, C], F32, kind="ExternalOutput")
    DBG = os.environ.get("K_DBG", "0") == "1"
    if DBG:
        te_dbg = nc.dram_tensor("te_dbg", [128, NLOC], BF16,
                                kind="ExternalOutput")
        hc_dbg = nc.dram_tensor("hc_dbg", [128, NLOC], F32,
                                kind="ExternalOutput")

    tab_loc_a = [nc.dram_tensor(f"tab{l}_loc_a", [HALF_LOC, C], BF16)
                 for l in (1, 2)]
    tab_loc_b = [nc.dram_tensor(f"tab{l}_loc_b", [NLOC - HALF_LOC, C], BF16)
                 for l in (1, 2)]
    tab_glo = [nc.dram_tensor(f"tab{l}_glo", [NLO, C], BF16,
                              addr_space="Shared") for l in (1, 2)]
    tab_ghi = [nc.dram_tensor(f"tab{l}_ghi", [NHI, C], BF16,
                              addr_space="Shared") for l in (1, 2)]
    COPYTAB = os.environ.get("K_COPYTAB", "0") == "1"
    AG2LATE = os.environ.get("K_AG2LATE", "0") == "1"
    if COPYTAB:
        tab_glo_l = [nc.dram_tensor(f"tab{l}_glo_l", [NLO, C], BF16)
                     for l in (1, 2)]
        tab_ghi_l = [nc.dram_tensor(f"tab{l}_ghi_l", [NHI, C], BF16)
                     for l in (1, 2)]
    bn_in = nc.dram_tensor("bn_in", [128, 2], F32)
    bn_out = nc.dram_tensor("bn_out", [128, 2], F32, addr_space="Shared")

    RG = [list(range(NCORES))]

    with tile.TileContext(nc) as tc:
        res_cm = tc.tile_pool(name="res", bufs=1)
        res = res_cm.__enter__()

        # ---- resident tiles ----
        nf_t = res.tile([128, NLOC], BF16, name="nf_t")
        nc.sync.dma_start(nf_t[:], nf_fm[:])
        te_t = res.tile([128, NLOC], BF16, name="te_t")
        dinvb_t = res.tile([128, NLOC], F32, name="dinvb_t")
        nc.sync.dma_start(dinvb_t[:], dinvb[:])
        dinv_nm_t = res.tile([128, NW], F32, name="dinv_nm_t")
        nc.sync.dma_start(dinv_nm_t[:], dinv_nm[:])
        tabfm = [res.tile([128, NLOC], BF16, name=f"tabfm{l}") for l in (1, 2)]
        Hpre_t = res.tile([128, NLOC], F32, name="Hpre_t")
        mask_t = res.tile([1, NLOC], BF16, name="mask_t")
        nc.sync.dma_start(mask_t[:], mask_row[:])

        w_t = {}
        for nm in cn:
            shape = list(cn[nm].shape)
            w_t[nm] = res.tile(shape, cn[nm].dtype, name=f"w_{nm}")
            nc.sync.dma_start(w_t[nm][:], cn[nm][:])
        ones2 = res.tile([2, 512], BF16, name="ones2")
        nc.vector.memset(ones2[:], 1.0)
        zero_col = res.tile([128, 1], F32, name="zero_col")
        nc.vector.memset(zero_col[:], 0.0)
        part_s = res.tile([128, NW], F32, name="part_s")
        part_q = res.tile([128, NW], F32, name="part_q")
        msum = res.tile([128, 2], F32, name="msum")
        bnred = res.tile([128, 2], F32, name="bnred")
        mean_c = res.tile([128, 1], F32, name="mean_c")
        istd_c = res.tile([128, 1], F32, name="istd_c")

        # ================= phase 1: t_embed via Sin =================
        with tc.tile_pool(name="p1", bufs=1) as p1:
            CH = 1568
            for lo in range(0, NLOC, CH):
                cs = slice(lo, lo + CH)
                tsr = p1.tile([128, CH], F32, name="tsr", tag="tsr", bufs=2)
                nc.sync.dma_start(tsr[:], ts_rep[:, cs])
                u = p1.tile([128, CH], F32, name="u", tag="u", bufs=1)
                nc.vector.tensor_scalar(u[:], tsr[:],
                                        w_t["freq2_col"][:],
                                        w_t["phq_col"][:],
                                        op0=AL.mult, op1=AL.add)
                ui = p1.tile([128, CH], I32, name="ui", tag="ui", bufs=1)
                nc.vector.tensor_copy(ui[:], u[:])
                uf = p1.tile([128, CH], F32, name="uf", tag="uf", bufs=1)
                nc.vector.tensor_copy(uf[:], ui[:])
                f = p1.tile([128, CH], F32, name="f", tag="f", bufs=1)
                nc.vector.tensor_tensor(f[:], u[:], uf[:], op=AL.subtract)
                st = p1.tile([128, CH], F32, name="st", tag="st", bufs=1)
                nc.vector.tensor_scalar(st[:], f[:], 0.5, None, op0=AL.is_ge)
                g = p1.tile([128, CH], F32, name="g", tag="g", bufs=1)
                nc.vector.tensor_tensor(g[:], f[:], st[:], op=AL.subtract)
                nc.scalar.activation(te_t[:, cs], g[:], AF.Sin,
                                     bias=0.0, scale=float(2 * np.pi))
                if DBG:
                    nc.scalar.dma_start(te_dbg[:, cs], te_t[:, cs])

        # ================= tab production helper =================
        def tab_prod(l, w, produce, tp, tps):
            """produce(pt, ws): node-major [n, c] psum for window w."""
            ws = slice(w * 128, (w + 1) * 128)
            pt = tps.tile([128, 128], F32, name="pt", tag="pt", bufs=1)
            produce(pt, ws)
            ot = tp.tile([128, 128], BF16, name="ot", tag="ot", bufs=3)
            nc.vector.tensor_scalar(ot[:], pt[:], dinv_nm_t[:, w:w + 1],
                                    None, op0=AL.mult)
            if w < NWA:
                nc.scalar.dma_start(tab_loc_a[l - 1][ws, :], ot[:])
            else:
                ws2 = slice((w - NWA) * 128, (w - NWA + 1) * 128)
                nc.scalar.dma_start(tab_loc_b[l - 1][ws2, :], ot[:])
            ptf = tps.tile([128, 128], BF16, name="ptf", tag="ptf", bufs=1)
            nc.tensor.transpose(ptf[:], ot[:], w_t["ident_b"][:])
            nc.vector.tensor_copy(tabfm[l - 1][:, ws], ptf[:])

        def fire_ag(l, part):
            if part == 0:
                nc.gpsimd.collective_compute(
                    "AllGather", AL.bypass, replica_groups=RG,
                    ins=[tab_loc_a[l - 1][:]], outs=[tab_glo[l - 1][:]])
                if COPYTAB:
                    nc.sync.dma_start(tab_glo_l[l - 1][:], tab_glo[l - 1][:])
            else:
                nc.gpsimd.collective_compute(
                    "AllGather", AL.bypass, replica_groups=RG,
                    ins=[tab_loc_b[l - 1][:]], outs=[tab_ghi[l - 1][:]])
                if COPYTAB:
                    nc.sync.dma_start(tab_ghi_l[l - 1][:], tab_ghi[l - 1][:])

        # ---- tab1 ----
        def prod1(pt, ws):
            nc.tensor.matmul(pt[:], nf_t[:, ws], w_t["R1a"][:],
                             start=True, stop=False)
            nc.tensor.matmul(pt[:], te_t[:, ws], w_t["R1b"][:],
                             start=False, stop=False)
            nc.tensor.matmul(pt[:], ones2[:, 0:128], w_t["tab1_bias2"][:],
                             start=False, stop=True)

        with tc.tile_pool(name="tab1", bufs=1) as tp, \
             tc.tile_pool(name="tab1ps", bufs=1, space="PSUM") as tps:
            for w in range(NW):
                tab_prod(1, w, prod1, tp, tps)
                if w == NWA - 1:
                    fire_ag(1, 0)
            fire_ag(1, 1)

        # ================= GRU window helper =================
        def gru_win(l, Hcb, xf, xfb, gp, gps):
            def mm2(wi, wh, tag):
                pi = gps.tile([128, 128], F32, name=tag, tag="pi", bufs=2)
                nc.tensor.matmul(pi[:], w_t[wi][:], Hcb[:],
                                 start=True, stop=False)
                nc.tensor.matmul(pi[:], w_t[wh][:], xfb[:],
                                 start=False, stop=True)
                return pi

            smr = mm2(f"g{l}Wir", f"g{l}Whr", "smr")
            r = gp.tile([128, 128], F32, name="r", tag="r", bufs=2)
            nc.scalar.activation(r[:], smr[:], AF.Sigmoid,
                                 bias=w_t[f"g{l}brz_r"][:])
            smz = mm2(f"g{l}Wiz", f"g{l}Whz", "smz")
            z = gp.tile([128, 128], F32, name="z", tag="z", bufs=2)
            nc.scalar.activation(z[:], smz[:], AF.Sigmoid,
                                 bias=w_t[f"g{l}brz_z"][:])
            pin = gps.tile([128, 128], F32, name="pin", tag="pi", bufs=2)
            nc.tensor.matmul(pin[:], w_t[f"g{l}Win"][:], Hcb[:],
                             start=True, stop=True)
            phn = gps.tile([128, 128], F32, name="phn", tag="pi", bufs=2)
            nc.tensor.matmul(phn[:], w_t[f"g{l}Whn"][:], xfb[:],
                             start=True, stop=True)
            hn = gp.tile([128, 128], F32, name="hn", tag="hn", bufs=2)
            nc.vector.tensor_scalar(hn[:], phn[:], w_t[f"g{l}bhn"][:],
                                    None, op0=AL.add)
            rn = gp.tile([128, 128], F32, name="rn", tag="rn", bufs=2)
            nc.vector.tensor_tensor(rn[:], r[:], hn[:], op=AL.mult)
            t2 = gp.tile([128, 128], F32, name="t2", tag="t2", bufs=2)
            nc.vector.tensor_tensor(t2[:], pin[:], rn[:], op=AL.add)
            ng = gp.tile([128, 128], F32, name="ng", tag="ng", bufs=2)
            nc.scalar.activation(ng[:], t2[:], AF.Tanh,
                                 bias=w_t[f"g{l}bin"][:])
            d = gp.tile([128, 128], F32, name="d", tag="d", bufs=2)
            nc.vector.tensor_tensor(d[:], xf[:], ng[:], op=AL.subtract)
            zd = gp.tile([128, 128], F32, name="zd", tag="zd", bufs=2)
            nc.vector.tensor_tensor(zd[:], z[:], d[:], op=AL.mult)
            H = gp.tile([128, 128], F32, name="H", tag="H", bufs=2)
            nc.vector.tensor_tensor(H[:], ng[:], zd[:], op=AL.add)
            return H

        # ================= conv layer =================
        def conv_layer(l, b_col, xf_dram, xfb_dram, fin):
            tglo = tab_glo_l[l - 1] if COPYTAB else tab_glo[l - 1]
            tghi = tab_ghi_l[l - 1] if COPYTAB else tab_ghi[l - 1]
            with tc.tile_pool(name=f"cv{l}", bufs=1) as gp, \
                 tc.tile_pool(name=f"cv{l}ps", bufs=1, space="PSUM") as cps:
                for w in range(NW):
                    ws = slice(w * 128, (w + 1) * 128)
                    t_lo, t_hi = tcnt[w][0], tcnt[w][1]
                    base8 = [off_slots[w][0] * 8, off_slots[w][1] * 8]

                    it_lo = gp.tile([128, TMAX * 8], I16, name="it_lo",
                                    tag="it_lo", bufs=6)
                    nc.sync.dma_start(
                        it_lo[:, :t_lo * 8],
                        idx_all[:, base8[0]:base8[0] + t_lo * 8])
                    it_hi = gp.tile([128, TMAX * 8], I16, name="it_hi",
                                    tag="it_hi", bufs=6)
                    nc.sync.dma_start(
                        it_hi[:, :t_hi * 8],
                        idx_all[:, base8[1]:base8[1] + t_hi * 8])
                    pos_lo = gp.tile([128, 128], F32, name="pos_lo",
                                     tag="pos_lo", bufs=6)
                    nc.sync.dma_start(
                        pos_lo[:], pos_all[:, (w * 2) * 128:(w * 2 + 1) * 128])
                    pos_hi = gp.tile([128, 128], F32, name="pos_hi",
                                     tag="pos_hi", bufs=6)
                    nc.sync.dma_start(
                        pos_hi[:],
                        pos_all[:, (w * 2 + 1) * 128:(w * 2 + 2) * 128])

                    glo = gp.tile([128, TMAX, 128], BF16, name="glo",
                                  tag="glo", bufs=4)
                    ghi = gp.tile([128, TMAX, 128], BF16, name="ghi",
                                  tag="ghi", bufs=4)
                    if w < 4:
                        nc.vector.memset(glo[:], 0.0)
                        nc.vector.memset(ghi[:], 0.0)
                    nc.gpsimd.dma_gather(
                        glo[:, :t_lo, :], tglo[:], it_lo[:, :t_lo * 8],
                        t_lo * 128, t_lo * 128, 128,
                        single_packet=False, queue_num=(2 * w) % NQ)
                    nc.gpsimd.dma_gather(
                        ghi[:, :t_hi, :], tghi[:], it_hi[:, :t_hi * 8],
                        t_hi * 128, t_hi * 128, 128,
                        single_packet=False, queue_num=(2 * w + 1) % NQ)

                    ps = cps.tile([128, 128], F32, name="ps", tag="ps",
                                  bufs=2)
                    S_lo = gp.tile([128, TMAX, 128], BF16, name="S_lo",
                                   tag="S_lo", bufs=2)
                    S_hi = gp.tile([128, TMAX, 128], BF16, name="S_hi",
                                   tag="S_hi", bufs=2)
                    for t in range(t_lo):
                        nc.vector.tensor_scalar(
                            S_lo[:, t, :], pos_lo[:],
                            w_t["iotaT"][:, t:t + 1], None, op0=AL.is_ge)
                    for t in range(t_hi):
                        nc.vector.tensor_scalar(
                            S_hi[:, t, :], pos_hi[:],
                            w_t["iotaT"][:, t:t + 1], None, op0=AL.is_ge)
                    for t in range(t_lo):
                        nc.tensor.matmul(ps[:], glo[:, t, :], S_lo[:, t, :],
                                         start=(t == 0), stop=False)
                    for t in range(t_hi):
                        nc.tensor.matmul(ps[:], ghi[:, t, :], S_hi[:, t, :],
                                         start=False, stop=(t == t_hi - 1))

                    # epilogue: diff -> +selfloop -> *dinv -> +b
                    pcp = gp.tile([128, 128], F32, name="pcp", tag="pcp",
                                  bufs=2)
                    nc.vector.tensor_copy(pcp[:], ps[:])
                    d0 = gp.tile([128, 128], F32, name="d0", tag="d0", bufs=2)
                    nc.vector.tensor_copy(d0[:, 0:1], pcp[:, 0:1])
                    nc.vector.tensor_tensor(d0[:, 1:128], pcp[:, 1:128],
                                            pcp[:, 0:127], op=AL.subtract)
                    d1 = gp.tile([128, 128], F32, name="d1", tag="d1", bufs=2)
                    nc.vector.tensor_tensor(d1[:], d0[:],
                                            tabfm[l - 1][:, ws], op=AL.add)
                    d2 = gp.tile([128, 128], F32, name="d2", tag="d2", bufs=2)
                    nc.vector.tensor_tensor(d2[:], d1[:], dinvb_t[:, ws],
                                            op=AL.mult)
                    Hc = gp.tile([128, 128], F32, name="Hc", tag="Hc", bufs=2)
                    nc.vector.tensor_scalar(Hc[:], d2[:], b_col, None,
                                            op0=AL.add)
                    Hcb = gp.tile([128, 128], BF16, name="Hcb", tag="Hcb",
                                  bufs=2)
                    nc.vector.tensor_copy(Hcb[:], Hc[:])
                    if DBG and l == 1:
                        nc.scalar.dma_start(hc_dbg[:, ws], Hc[:])

                    xf = gp.tile([128, 128], F32, name="xf", tag="xf", bufs=3)
                    nc.sync.dma_start(xf[:], xf_dram[:, ws])
                    xfb = gp.tile([128, 128], BF16, name="xfb", tag="xfb",
                                  bufs=3)
                    nc.sync.dma_start(xfb[:], xfb_dram[:, ws])

                    H = gru_win(l, Hcb, xf, xfb, gp, cps)
                    fin(w, ws, H, gp, cps)

        # ---- layer 1 ----
        def fin1(w, ws, H, gp, gps):
            Hr = gp.tile([128, 128], F32, name="Hr", tag="Hr", bufs=2)
            nc.scalar.activation(Hr[:], H[:], AF.Relu, bias=zero_col[:])
            H1b = gp.tile([128, 128], BF16, name="H1b", tag="H1b", bufs=2)
            nc.vector.tensor_copy(H1b[:], Hr[:])
            # h1 out (transpose to node-major)
            ptr = gps.tile([128, 128], F32, name="ptr", tag="ptr", bufs=1)
            nc.tensor.transpose(ptr[:], Hr[:], w_t["ident_f"][:])
            ob = gp.tile([128, 128], F32, name="ob", tag="ob", bufs=3)
            nc.scalar.copy(ob[:], ptr[:])
            nc.scalar.dma_start(h1_out[ws, :], ob[:])

            # tab2 production
            def prod2(pt, ws_):
                nc.tensor.matmul(pt[:], H1b[:], w_t["W2T"][:],
                                 start=True, stop=True)
            tab_prod(2, w, prod2, gp, gps)
            if not AG2LATE:
                if w == NWA - 1:
                    fire_ag(2, 0)
                if w == NW - 1:
                    fire_ag(2, 1)

        if PH >= 2:
            conv_layer(1, w_t["b1_col"][:], xp1_fm, xp1_fmb, fin1)
            if AG2LATE:
                fire_ag(2, 0)
                fire_ag(2, 1)

        # ---- layer 2 ----
        def fin2(w, ws, H, gp, gps):
            pk = gps.tile([128, 128], F32, name="pk", tag="pk", bufs=1)
            nc.tensor.matmul(pk[:], w_t["S1a"][:], nf_t[:, ws],
                             start=True, stop=False)
            nc.tensor.matmul(pk[:], w_t["S1b"][:], te_t[:, ws],
                             start=False, stop=False)
            nc.tensor.matmul(pk[:], w_t["skip_bias2"][:], ones2[:, 0:128],
                             start=False, stop=True)
            nc.vector.tensor_tensor(Hpre_t[:, ws], H[:], pk[:], op=AL.add)
            # BN partial stats
            pm = gps.tile([128, 128], F32, name="pm", tag="pm", bufs=1)
            nc.tensor.matmul(pm[:], ones2[0:1, 0:128], mask_t[:, ws],
                             start=True, stop=True)
            hm = gp.tile([128, 128], F32, name="hm", tag="hm", bufs=2)
            nc.vector.tensor_tensor(hm[:], Hpre_t[:, ws], pm[:], op=AL.mult)
            nc.vector.tensor_reduce(part_s[:, w:w + 1], hm[:],
                                    axis=mybir.AxisListType.X, op=AL.add)
            sqs = gp.tile([128, 128], F32, name="sqs", tag="sqs", bufs=2)
            nc.scalar.activation(sqs[:], hm[:], AF.Square, bias=0.0,
                                 accum_out=part_q[:, w:w + 1])

        if PH >= 3:
            conv_layer(2, w_t["b2_col"][:], xp2_fm, xp2_fmb, fin2)
        else:
            nc.vector.memset(Hpre_t[:], 0.0)
            nc.vector.memset(part_s[:], 0.0)
            nc.vector.memset(part_q[:], 0.0)
            z1 = res.tile([128, 128], F32, name="z1")
            nc.vector.memset(z1[:], 0.0)
            if PH < 2:
                for w in range(NW):
                    nc.sync.dma_start(h1_out[w * 128:(w + 1) * 128, :], z1[:])

        # ================= BatchNorm finale =================
        with tc.tile_pool(name="bn", bufs=1) as bp, \
             tc.tile_pool(name="bnps", bufs=1, space="PSUM") as bps:
            nc.vector.tensor_reduce(msum[:, 0:1], part_s[:],
                                    axis=mybir.AxisListType.X, op=AL.add)
            nc.vector.tensor_reduce(msum[:, 1:2], part_q[:],
                                    axis=mybir.AxisListType.X, op=AL.add)
            nc.scalar.dma_start(bn_in[:], msum[:])
            nc.gpsimd.collective_compute(
                "AllReduce", AL.add, replica_groups=RG,
                ins=[bn_in[:]], outs=[bn_out[:]])
            nc.sync.dma_start(bnred[:], bn_out[:])
            nc.vector.tensor_scalar(mean_c[:], bnred[:, 0:1], 1.0 / N, None,
                                    op0=AL.mult)
            m2 = bp.tile([128, 1], F32, name="m2")
            nc.vector.tensor_tensor(m2[:], mean_c[:], mean_c[:], op=AL.mult)
            v1 = bp.tile([128, 1], F32, name="v1")
            nc.vector.tensor_scalar(v1[:], bnred[:, 1:2], 1.0 / N, None,
                                    op0=AL.mult)
            v2 = bp.tile([128, 1], F32, name="v2")
            nc.vector.tensor_tensor(v2[:], v1[:], m2[:], op=AL.subtract)
            v3 = bp.tile([128, 1], F32, name="v3")
            nc.vector.tensor_scalar(v3[:], v2[:], 1e-5, None, op0=AL.add)
            v4 = bp.tile([128, 1], F32, name="v4")
            nc.scalar.activation(v4[:], v3[:], AF.Sqrt, bias=zero_col[:])
            nc.vector.reciprocal(istd_c[:], v4[:])
            for w in range(NW):
                ws = slice(w * 128, (w + 1) * 128)
                hn_ = bp.tile([128, 128], F32, name="hn_", tag="hn_", bufs=2)
                nc.vector.tensor_scalar(hn_[:], Hpre_t[:, ws],
                                        mean_c[:], istd_c[:],
                                        op0=AL.subtract, op1=AL.mult)
                ptr = bps.tile([128, 128], F32, name="ptr", tag="ptr", bufs=2)
                nc.tensor.transpose(ptr[:], hn_[:], w_t["ident_f"][:])
                ob = bp.tile([128, 128], F32, name="ob", tag="ob", bufs=3)
                nc.scalar.copy(ob[:], ptr[:])
                nc.scalar.dma_start(h2_out[ws, :], ob[:])

        res_cm.__exit__(None, None, None)
    return nc


# ---------------------------------------------------------------------------
# entry point
# ---------------------------------------------------------------------------

def _install_ntff_hook():
    """Install antenv.axon_hooks (missing in this image) for trace=True."""
    import sys
    import types
    try:
        import antenv
        if getattr(antenv, "axon_hooks", None) is not None:
            return
        from trn_agent_boot.trn_boot import _ntff_profile_via_ctypes
        hook = _ntff_profile_via_ctypes("/opt/axon/libaxon_pjrt.so")
        mod = types.ModuleType("antenv.axon_hooks")
        mod.set_axon_ntff_profile_hook = lambda h: None
        mod.get_axon_ntff_profile_hook = lambda: hook
        sys.modules["antenv.axon_hooks"] = mod
        antenv.axon_hooks = mod
    except Exception:
        pass


def kernel(**inputs):
    global LAST_EXEC_NS
    per_core, static = _preprocess(inputs)

    nc = bacc.Bacc("TRN2", target_bir_lowering=False, debug=False,
                   num_devices=NCORES, num_swdge_queues=4)
    _build(nc, static)
    nc.compile()

    in_maps = [per_core[k] for k in range(NCORES)]
    trace = os.environ.get("KERNEL_TRACE", "0") == "1"
    if trace:
        _install_ntff_hook()
    res = run_bass_kernel_spmd(nc, in_maps, list(range(NCORES)), trace=trace)
    LAST_EXEC_NS = res.exec_time_ns
    global LAST_RESULTS
    LAST_RESULTS = res.results

    H1 = np.zeros((N, C), np.float32)
    H2 = np.zeros((N, C), np.float32)
    for k in range(NCORES):
        lo, hi_ = k * NLOC, min((k + 1) * NLOC, N)
        if lo >= N:
            break
        nrow = hi_ - lo
        H1[lo:hi_] = res.results[k]["h1_out"][:nrow]
        H2[lo:hi_] = res.results[k]["h2_out"][:nrow]
    return (H1, H2)


# revision 16
# speedup vs baseline: 1.3133x; 1.2702x over previous
"""Trainium2 Bass kernel for nn_DynamicGCNModel (2-layer GCN+GRU, 50k nodes,
1.6M edges, C=128) on 8 NeuronCores.

v5 design:
- Nodes sharded 6272/core; edges partitioned by 256-node destination window
  (24x256 + 1x128), sorted by (window, src-half, dst-local).
- Per-edge source rows fetched with dma_gather in 16-tile (<=2048 idx) chunks,
  4 SWDGE queues round-robin so up to 4 descriptor-generations overlap on the
  4 Q7 core pairs.
- Segment-sum via "staircase" matmul: S[e,n] = (pos_n >= e) built with one
  tensor_scalar is_ge per 128-edge tile ([128, W] wide); PSUM accumulates
  P[c,n]; adjacent-column differencing recovers per-node sums. Self-loops
  applied densely from the feature-major table.
- The two AllGather halves ARE the lo/hi gather tables; layer-2's first AG
  half fires mid-conv1. GRU / tab2 / skip / BN-stats run in 512-wide chunks.
- TimeEncode via scalar-engine Sin (robust range reduction).
- Input DMAs ride the Sync HWDGE ring; output DMAs ride the Scalar ring so
  input prefetch is never blocked behind compute.
"""

import os

import numpy as np
import ml_dtypes

import concourse.bass as bass
import concourse.bacc as bacc
import concourse.mybir as mybir
import concourse.tile as tile
from concourse.bass_utils import run_bass_kernel_spmd

BF = ml_dtypes.bfloat16
F32 = mybir.dt.float32
BF16 = mybir.dt.bfloat16
I16 = mybir.dt.int16
I32 = mybir.dt.int32
AL = mybir.AluOpType
AF = mybir.ActivationFunctionType

N = 50000
NV = 50176
C = 128
NCORES = 8
NLOC = NV // NCORES     # 6272
WIN = 256
NW2 = (NLOC + WIN - 1) // WIN   # 25 (last window 128 wide)
HALF_LOC = 3072
NLO = NCORES * HALF_LOC           # 24576 rows in lo table
NHI = NCORES * (NLOC - HALF_LOC)  # 25600 rows in hi table
GC = 16                 # tiles per gather chunk

LAST_EXEC_NS = None
LAST_RESULTS = None


def _wwid(w):
    return min(WIN, NLOC - w * WIN)


# ---------------------------------------------------------------------------
# host preprocessing
# ---------------------------------------------------------------------------

def _hilo(a):
    a = np.asarray(a, np.float32)
    hi = a.astype(BF)
    lo = (a - hi.astype(np.float32)).astype(BF)
    return np.stack([hi, lo], 0)


def _preprocess(inp):
    src = np.asarray(inp["edge_index"][0], np.int64)
    dst = np.asarray(inp["edge_index"][1], np.int64)

    # degree includes the self-loop (handled densely on device)
    deg = np.bincount(dst, minlength=NV).astype(np.float32) + 1.0
    deg[N:] = 0.0
    dinv = np.where(deg > 0, 1.0 / np.sqrt(np.maximum(deg, 1.0)),
                    0.0).astype(np.float32)

    score = src // NLOC
    swithin = src % NLOC
    half = (swithin >= HALF_LOC).astype(np.int64)
    row16 = np.where(half == 0,
                     score * HALF_LOC + swithin,
                     score * (NLOC - HALF_LOC) + (swithin - HALF_LOC))

    dcore = dst // NLOC
    win = (dst % NLOC) // WIN
    nl = (dst % NLOC) - win * WIN

    order = np.lexsort((nl, half, win, dcore))
    dcore_s = dcore[order]
    win_s = win[order]
    half_s = half[order]
    nl_s = nl[order]
    row_s = row16[order]

    key = (dcore_s * NW2 + win_s) * 2 + half_s
    nkeys = NCORES * NW2 * 2
    cnt = np.bincount(key, minlength=nkeys).reshape(NCORES, NW2, 2)
    tcnt = np.ceil(cnt.max(axis=0) / 128.0).astype(np.int64)  # [NW2, 2]
    tcnt = np.maximum(tcnt, 1)
    TMAX = int(tcnt.max())
    off_slots = np.zeros((NW2, 2), np.int64)
    acc = 0
    for w in range(NW2):
        for h in range(2):
            off_slots[w, h] = acc
            acc += tcnt[w, h]
    TOTT = int(acc)

    starts = np.zeros(nkeys + 1, np.int64)
    np.cumsum(cnt.reshape(-1), out=starts[1:])
    pos_in_key = np.arange(len(key)) - starts[key]

    per_core = []
    for k in range(NCORES):
        sel = dcore_s == k
        w_ = win_s[sel]
        h_ = half_s[sel]
        p_ = pos_in_key[sel]
        r_ = row_s[sel]
        n_ = nl_s[sel]

        idx_arr = np.zeros((16, TOTT * 8), np.int16)
        j = off_slots[w_, h_] * 128 + p_
        idx_arr[j % 16, (j // 16)] = r_.astype(np.int16)

        # pos[n] per (w, h): (# edges with nl <= n) - 1
        pos = np.full((NW2, 2, WIN), -1.0, np.float32)
        cnt_wh = np.zeros((NW2, 2, WIN), np.int64)
        np.add.at(cnt_wh, (w_, h_, n_), 1)
        pos[:, :, :] = np.cumsum(cnt_wh, axis=2) - 1.0
        pos_rep = np.broadcast_to(
            pos.reshape(1, NW2 * 2 * WIN), (128, NW2 * 2 * WIN)).copy()

        lo, hi_ = k * NLOC, (k + 1) * NLOC
        d = dict(
            idx_all=np.tile(idx_arr, (8, 1)),
            pos_all=pos_rep.astype(np.float32),
        )
        nrow = max(0, min(NLOC, N - lo))
        nfp = np.zeros((NLOC, C), np.float32)
        ts_p = np.zeros(NLOC, np.float32)
        xp1 = np.zeros((NLOC, C), np.float32)
        xp2 = np.zeros((NLOC, C), np.float32)
        if nrow > 0:
            nfp[:nrow] = np.asarray(inp["node_features"][lo:lo + nrow],
                                    np.float32)
            ts_p[:nrow] = np.asarray(inp["ts"][lo:lo + nrow],
                                     np.float32).reshape(-1)
            xp1[:nrow] = np.asarray(inp["x_prev1"][lo:lo + nrow], np.float32)
            xp2[:nrow] = np.asarray(inp["x_prev2"][lo:lo + nrow], np.float32)
        d["nf_fm"] = np.ascontiguousarray(nfp.T.astype(BF))
        d["ts_rep"] = np.broadcast_to(ts_p.reshape(1, NLOC),
                                      (128, NLOC)).copy()
        d["xp1_fm"] = np.ascontiguousarray(xp1.T)
        d["xp1_fmb"] = np.ascontiguousarray(xp1.T.astype(BF))
        d["xp2_fm"] = np.ascontiguousarray(xp2.T)
        d["xp2_fmb"] = np.ascontiguousarray(xp2.T.astype(BF))
        dv = dinv[lo:hi_]
        d["dinv_nm"] = np.ascontiguousarray(dv.reshape(NLOC // 128, 128).T)
        d["dinvb"] = np.broadcast_to(dv.reshape(1, NLOC), (128, NLOC)).copy()
        mask = np.zeros((1, NLOC), np.float32)
        mask[0, :nrow] = 1.0
        d["mask_row"] = mask.astype(BF)
        per_core.append(d)

    # shared consts
    freq = np.asarray(inp["basis_freq"], np.float64)
    freq2_col = (freq / (2 * np.pi)).astype(np.float32).reshape(C, 1)
    # te_stored = -cos(ts*freq + phase) = sin(2pi*(y - 0.25)); u = y + 0.75
    phq_col = (np.asarray(inp["phase"], np.float64) / (2 * np.pi)
               + 0.75).astype(np.float32).reshape(C, 1)

    mW = np.asarray(inp["merge_W"], np.float64)
    W1_ = np.asarray(inp["W1"], np.float64)
    W2_ = np.asarray(inp["W2"], np.float64)
    sW = np.asarray(inp["skip_W"], np.float64)
    M1 = mW.T @ W1_.T
    S1 = mW.T @ sW.T
    b_m = np.asarray(inp["merge_b"], np.float64)

    consts = dict(
        R1a=M1[:C].astype(BF), R1b=(-M1[C:]).astype(BF),
        S1a=S1[:C].astype(BF), S1b=(-S1[C:]).astype(BF),
        W2T=W2_.T.astype(BF),
        tab1_bias2=_hilo(b_m @ W1_.T).reshape(2, C),
        skip_bias2=_hilo(b_m @ sW.T +
                         np.asarray(inp["skip_b"], np.float64)).reshape(2, C),
        b1_col=np.asarray(inp["b1"], np.float32).reshape(C, 1),
        b2_col=np.asarray(inp["b2"], np.float32).reshape(C, 1),
        freq2_col=freq2_col, phq_col=phq_col,
        iotaT=(np.arange(128, dtype=np.float32).reshape(128, 1)
               + 128.0 * np.arange(TMAX, dtype=np.float32).reshape(1, TMAX)),
        ident_f=np.eye(128, dtype=np.float32),
        ident_b=np.eye(128, dtype=np.float32).astype(BF),
    )
    for l in (1, 2):
        Wih = np.asarray(inp[f"gru{l}_Wih"], np.float32)
        Whh = np.asarray(inp[f"gru{l}_Whh"], np.float32)
        bih = np.asarray(inp[f"gru{l}_bih"], np.float32)
        bhh = np.asarray(inp[f"gru{l}_bhh"], np.float32)
        for gi, gate in enumerate("rzn"):
            consts[f"g{l}Wi{gate}"] = Wih[gi * C:(gi + 1) * C].T.astype(BF)
            consts[f"g{l}Wh{gate}"] = Whh[gi * C:(gi + 1) * C].T.astype(BF)
        consts[f"g{l}brz_r"] = (bih[0:C] + bhh[0:C]).reshape(C, 1)
        consts[f"g{l}brz_z"] = (bih[C:2 * C] + bhh[C:2 * C]).reshape(C, 1)
        consts[f"g{l}bin"] = bih[2 * C:].reshape(C, 1)
        consts[f"g{l}bhn"] = bhh[2 * C:].reshape(C, 1)

    for d in per_core:
        d.update(consts)

    static = dict(tcnt=tcnt.tolist(), off_slots=off_slots.tolist(),
                  TOTT=TOTT, TMAX=TMAX)
    return per_core, static


# ---------------------------------------------------------------------------
# bass program
# ---------------------------------------------------------------------------

def _build(nc, static):
    PH = int(os.environ.get("K_PH", "9"))
    NQ = int(os.environ.get("K_NQ", "4"))
    COPYTAB = os.environ.get("K_COPYTAB", "1") == "1"
    SPKT = os.environ.get("K_SPKT", "0") == "1"
    tcnt = static["tcnt"]
    off_slots = static["off_slots"]
    TOTT = static["TOTT"]
    TMAX = static["TMAX"]

    def din(name, shape, dt):
        return nc.dram_tensor(name, shape, dt, kind="ExternalInput")

    idx_all = din("idx_all", [128, TOTT * 8], I16)
    pos_all = din("pos_all", [128, NW2 * 2 * WIN], F32)
    nf_fm = din("nf_fm", [128, NLOC], BF16)
    ts_rep = din("ts_rep", [128, NLOC], F32)
    xp1_fm = din("xp1_fm", [128, NLOC], F32)
    xp1_fmb = din("xp1_fmb", [128, NLOC], BF16)
    xp2_fm = din("xp2_fm", [128, NLOC], F32)
    xp2_fmb = din("xp2_fmb", [128, NLOC], BF16)
    dinv_nm = din("dinv_nm", [128, NLOC // 128], F32)
    dinvb = din("dinvb", [128, NLOC], F32)
    mask_row = din("mask_row", [1, NLOC], BF16)

    cn = {}
    for nm, shape, dt in [
        ("R1a", [C, C], BF16), ("R1b", [C, C], BF16),
        ("S1a", [C, C], BF16), ("S1b", [C, C], BF16),
        ("W2T", [C, C], BF16),
        ("tab1_bias2", [2, C], BF16), ("skip_bias2", [2, C], BF16),
        ("b1_col", [C, 1], F32), ("b2_col", [C, 1], F32),
        ("freq2_col", [C, 1], F32), ("phq_col", [C, 1], F32),
        ("iotaT", [128, TMAX], F32),
        ("ident_f", [128, 128], F32), ("ident_b", [128, 128], BF16),
    ]:
        cn[nm] = din(nm, shape, dt)
    for l in (1, 2):
        for gate in "rzn":
            cn[f"g{l}Wi{gate}"] = din(f"g{l}Wi{gate}", [C, C], BF16)
            cn[f"g{l}Wh{gate}"] = din(f"g{l}Wh{gate}", [C, C], BF16)
        for nm in ("brz_r", "brz_z", "bin", "bhn"):
            cn[f"g{l}{nm}"] = din(f"g{l}{nm}", [C, 1], F32)

    h1_out = nc.dram_tensor("h1_out", [NLOC, C], F32, kind="ExternalOutput")
    h2_out = nc.dram_tensor("h2_out", [NLOC, C], F32, kind="ExternalOutput")

    tab_loc_a = [nc.dram_tensor(f"tab{l}_loc_a", [HALF_LOC, C], BF16)
                 for l in (1, 2)]
    tab_loc_b = [nc.dram_tensor(f"tab{l}_loc_b", [NLOC - HALF_LOC, C], BF16)
                 for l in (1, 2)]
    tab_glo = [nc.dram_tensor(f"tab{l}_glo", [NLO, C], BF16,
                              addr_space="Shared") for l in (1, 2)]
    tab_ghi = [nc.dram_tensor(f"tab{l}_ghi", [NHI, C], BF16,
                              addr_space="Shared") for l in (1, 2)]
    if COPYTAB:
        tab_glo_l = [nc.dram_tensor(f"tab{l}_glo_l", [NLO, C], BF16)
                     for l in (1, 2)]
        tab_ghi_l = [nc.dram_tensor(f"tab{l}_ghi_l", [NHI, C], BF16)
                     for l in (1, 2)]
    bn_in = nc.dram_tensor("bn_in", [128, 2], F32)
    bn_out = nc.dram_tensor("bn_out", [128, 2], F32, addr_space="Shared")

    RG = [list(range(NCORES))]
    gq = [0]  # round-robin gather queue counter

    with tile.TileContext(nc) as tc:
        res_cm = tc.tile_pool(name="res", bufs=1)
        res = res_cm.__enter__()

        # ---- resident tiles ----
        nf_t = res.tile([128, NLOC], BF16, name="nf_t")
        nc.sync.dma_start(nf_t[:], nf_fm[:])
        te_t = res.tile([128, NLOC], BF16, name="te_t")
        dinvb_t = res.tile([128, NLOC], F32, name="dinvb_t")
        nc.sync.dma_start(dinvb_t[:], dinvb[:])
        dinv_nm_t = res.tile([128, NLOC // 128], F32, name="dinv_nm_t")
        nc.sync.dma_start(dinv_nm_t[:], dinv_nm[:])
        tabfm = [res.tile([128, NLOC], BF16, name=f"tabfm{l}") for l in (1, 2)]
        Hpre_t = res.tile([128, NLOC], F32, name="Hpre_t")
        mask_t = res.tile([1, NLOC], BF16, name="mask_t")
        nc.sync.dma_start(mask_t[:], mask_row[:])

        w_t = {}
        for nm in cn:
            shape = list(cn[nm].shape)
            w_t[nm] = res.tile(shape, cn[nm].dtype, name=f"w_{nm}")
            nc.sync.dma_start(w_t[nm][:], cn[nm][:])
        ones2 = res.tile([2, 512], BF16, name="ones2")
        nc.vector.memset(ones2[:], 1.0)
        zero_col = res.tile([128, 1], F32, name="zero_col")
        nc.vector.memset(zero_col[:], 0.0)
        part_s = res.tile([128, 16], F32, name="part_s")
        part_q = res.tile([128, 16], F32, name="part_q")
        msum = res.tile([128, 2], F32, name="msum")
        bnred = res.tile([128, 2], F32, name="bnred")
        mean_c = res.tile([128, 1], F32, name="mean_c")
        istd_c = res.tile([128, 1], F32, name="istd_c")

        # ================= phase 1: t_embed via Sin =================
        with tc.tile_pool(name="p1", bufs=1) as p1:
            CH = 1568
            for lo in range(0, NLOC, CH):
                cs = slice(lo, lo + CH)
                tsr = p1.tile([128, CH], F32, name="tsr", tag="tsr", bufs=2)
                nc.sync.dma_start(tsr[:], ts_rep[:, cs])
                u = p1.tile([128, CH], F32, name="u", tag="u", bufs=1)
                nc.vector.tensor_scalar(u[:], tsr[:],
                                        w_t["freq2_col"][:],
                                        w_t["phq_col"][:],
                                        op0=AL.mult, op1=AL.add)
                ui = p1.tile([128, CH], I32, name="ui", tag="ui", bufs=1)
                nc.vector.tensor_copy(ui[:], u[:])
                uf = p1.tile([128, CH], F32, name="uf", tag="uf", bufs=1)
                nc.vector.tensor_copy(uf[:], ui[:])
                f = p1.tile([128, CH], F32, name="f", tag="f", bufs=1)
                nc.vector.tensor_tensor(f[:], u[:], uf[:], op=AL.subtract)
                st = p1.tile([128, CH], F32, name="st", tag="st", bufs=1)
                nc.vector.tensor_scalar(st[:], f[:], 0.5, None, op0=AL.is_ge)
                g = p1.tile([128, CH], F32, name="g", tag="g", bufs=1)
                nc.vector.tensor_tensor(g[:], f[:], st[:], op=AL.subtract)
                nc.scalar.activation(te_t[:, cs], g[:], AF.Sin,
                                     bias=0.0, scale=float(2 * np.pi))

        # ================= tab production (per 128 rows) ==============
        def tab_prod(l, r, produce, tp, tps):
            """produce(pt, rs): node-major [n, c] psum for 128-row block r."""
            rs = slice(r * 128, (r + 1) * 128)
            pt = tps.tile([128, 128], F32, name="pt", tag="pt", bufs=1)
            produce(pt, rs)
            ot = tp.tile([128, 128], BF16, name="ot", tag="ot", bufs=3)
            nc.vector.tensor_scalar(ot[:], pt[:], dinv_nm_t[:, r:r + 1],
                                    None, op0=AL.mult)
            if r * 128 < HALF_LOC:
                nc.scalar.dma_start(tab_loc_a[l - 1][rs, :], ot[:])
            else:
                rs2 = slice(r * 128 - HALF_LOC, (r + 1) * 128 - HALF_LOC)
                nc.scalar.dma_start(tab_loc_b[l - 1][rs2, :], ot[:])
            ptf = tps.tile([128, 128], BF16, name="ptf", tag="ptf", bufs=1)
            nc.tensor.transpose(ptf[:], ot[:], w_t["ident_b"][:])
            nc.vector.tensor_copy(tabfm[l - 1][:, rs], ptf[:])

        def fire_ag(l, part):
            if part == 0:
                nc.gpsimd.collective_compute(
                    "AllGather", AL.bypass, replica_groups=RG,
                    ins=[tab_loc_a[l - 1][:]], outs=[tab_glo[l - 1][:]])
                if COPYTAB:
                    nc.sync.dma_start(tab_glo_l[l - 1][:], tab_glo[l - 1][:])
            else:
                nc.gpsimd.collective_compute(
                    "AllGather", AL.bypass, replica_groups=RG,
                    ins=[tab_loc_b[l - 1][:]], outs=[tab_ghi[l - 1][:]])
                if COPYTAB:
                    nc.sync.dma_start(tab_ghi_l[l - 1][:], tab_ghi[l - 1][:])

        # ---- tab1 ----
        def prod1(pt, rs):
            nc.tensor.matmul(pt[:], nf_t[:, rs], w_t["R1a"][:],
                             start=True, stop=False)
            nc.tensor.matmul(pt[:], te_t[:, rs], w_t["R1b"][:],
                             start=False, stop=False)
            nc.tensor.matmul(pt[:], ones2[:, 0:128], w_t["tab1_bias2"][:],
                             start=False, stop=True)

        with tc.tile_pool(name="tab1", bufs=1) as tp, \
             tc.tile_pool(name="tab1ps", bufs=1, space="PSUM") as tps:
            for r in range(NLOC // 128):
                tab_prod(1, r, prod1, tp, tps)
                if r == HALF_LOC // 128 - 1:
                    fire_ag(1, 0)
            fire_ag(1, 1)

        # ================= GRU chunk (512 cols) =================
        def gru_chunk(l, Hcb, xf, xfb, n, gp, gps):
            def mm2(wi, wh, tag):
                pi = gps.tile([128, 512], F32, name=tag, tag="pi", bufs=2)
                nc.tensor.matmul(pi[:, :n], w_t[wi][:], Hcb[:, :n],
                                 start=True, stop=False)
                nc.tensor.matmul(pi[:, :n], w_t[wh][:], xfb[:, :n],
                                 start=False, stop=True)
                return pi

            smr = mm2(f"g{l}Wir", f"g{l}Whr", "smr")
            r = gp.tile([128, 512], F32, name="r", tag="r", bufs=2)
            nc.scalar.activation(r[:, :n], smr[:, :n], AF.Sigmoid,
                                 bias=w_t[f"g{l}brz_r"][:])
            smz = mm2(f"g{l}Wiz", f"g{l}Whz", "smz")
            z = gp.tile([128, 512], F32, name="z", tag="z", bufs=2)
            nc.scalar.activation(z[:, :n], smz[:, :n], AF.Sigmoid,
                                 bias=w_t[f"g{l}brz_z"][:])
            pin = gps.tile([128, 512], F32, name="pin", tag="pi", bufs=2)
            nc.tensor.matmul(pin[:, :n], w_t[f"g{l}Win"][:], Hcb[:, :n],
                             start=True, stop=True)
            phn = gps.tile([128, 512], F32, name="phn", tag="pi", bufs=2)
            nc.tensor.matmul(phn[:, :n], w_t[f"g{l}Whn"][:], xfb[:, :n],
                             start=True, stop=True)
            hn = gp.tile([128, 512], F32, name="hn", tag="hn", bufs=1)
            nc.vector.tensor_scalar(hn[:, :n], phn[:, :n],
                                    w_t[f"g{l}bhn"][:], None, op0=AL.add)
            rn = gp.tile([128, 512], F32, name="rn", tag="rn", bufs=1)
            nc.vector.tensor_tensor(rn[:, :n], r[:, :n], hn[:, :n],
                                    op=AL.mult)
            t2 = gp.tile([128, 512], F32, name="t2", tag="t2", bufs=1)
            nc.vector.tensor_tensor(t2[:, :n], pin[:, :n], rn[:, :n],
                                    op=AL.add)
            ng = gp.tile([128, 512], F32, name="ng", tag="ng", bufs=2)
            nc.scalar.activation(ng[:, :n], t2[:, :n], AF.Tanh,
                                 bias=w_t[f"g{l}bin"][:])
            d = gp.tile([128, 512], F32, name="d", tag="d", bufs=1)
            nc.vector.tensor_tensor(d[:, :n], xf[:, :n], ng[:, :n],
                                    op=AL.subtract)
            zd = gp.tile([128, 512], F32, name="zd", tag="zd", bufs=1)
            nc.vector.tensor_tensor(zd[:, :n], z[:, :n], d[:, :n],
                                    op=AL.mult)
            H = gp.tile([128, 512], F32, name="H", tag="H", bufs=2)
            nc.vector.tensor_tensor(H[:, :n], ng[:, :n], zd[:, :n],
                                    op=AL.add)
            return H

        # ================= conv layer =================
        def conv_layer(l, b_col, xf_dram, xfb_dram, fin):
            if COPYTAB:
                tglo, tghi = tab_glo_l[l - 1], tab_ghi_l[l - 1]
            else:
                tglo, tghi = tab_glo[l - 1], tab_ghi[l - 1]
            with tc.tile_pool(name=f"cv{l}", bufs=1) as gp, \
                 tc.tile_pool(name=f"cv{l}ps", bufs=1, space="PSUM") as cps:
                Hcb_ch = None
                xf = xfb = None
                for w in range(NW2):
                    ww = _wwid(w)
                    wsl = slice(w * WIN, w * WIN + ww)
                    # gather chunks for both halves
                    gtiles = {}   # (h, chunk) -> tile
                    for h, tab in ((0, tglo), (1, tghi)):
                        t_n = tcnt[w][h]
                        for j in range(0, t_n, GC):
                            ct = min(GC, t_n - j)
                            base8 = (off_slots[w][h] + j) * 8
                            it = gp.tile([128, GC * 8], I16, name="it",
                                         tag="it", bufs=8)
                            nc.sync.dma_start(
                                it[:, :ct * 8],
                                idx_all[:, base8:base8 + ct * 8])
                            gt = gp.tile([128, GC, 128], BF16, name="gt",
                                         tag="gt", bufs=6)
                            if w == 0 and j == 0:
                                nc.vector.memset(gt[:], 0.0)
                            q = gq[0] % NQ
                            gq[0] += 1
                            nc.gpsimd.dma_gather(
                                gt[:, :ct, :], tab[:], it[:, :ct * 8],
                                ct * 128, ct * 128, 128,
                                single_packet=SPKT, queue_num=q)
                            gtiles[(h, j // GC)] = gt
                    pos_lo = gp.tile([128, WIN], F32, name="pos_lo",
                                     tag="pos_lo", bufs=3)
                    nc.sync.dma_start(
                        pos_lo[:, :ww],
                        pos_all[:, (w * 2) * WIN:(w * 2) * WIN + ww])
                    pos_hi = gp.tile([128, WIN], F32, name="pos_hi",
                                     tag="pos_hi", bufs=3)
                    nc.sync.dma_start(
                        pos_hi[:, :ww],
                        pos_all[:, (w * 2 + 1) * WIN:(w * 2 + 1) * WIN + ww])

                    ps = cps.tile([128, WIN], F32, name="ps", tag="ps",
                                  bufs=2)
                    first = True
                    for h, pos in ((0, pos_lo), (1, pos_hi)):
                        t_n = tcnt[w][h]
                        for t in range(t_n):
                            S = gp.tile([128, WIN], BF16, name="S", tag="S",
                                        bufs=8)
                            nc.vector.tensor_scalar(
                                S[:, :ww], pos[:, :ww],
                                w_t["iotaT"][:, t:t + 1], None, op0=AL.is_ge)
                            gt = gtiles[(h, t // GC)]
                            last = (h == 1 and t == t_n - 1)
                            nc.tensor.matmul(ps[:, :ww], gt[:, t % GC, :],
                                             S[:, :ww], start=first,
                                             stop=last)
                            first = False

                    # epilogue: diff -> +selfloop -> *dinv -> +b
                    pcp = gp.tile([128, WIN], F32, name="pcp", tag="pcp",
                                  bufs=2)
                    nc.vector.tensor_copy(pcp[:, :ww], ps[:, :ww])
                    d0 = gp.tile([128, WIN], F32, name="d0", tag="d0",
                                 bufs=2)
                    nc.vector.tensor_copy(d0[:, 0:1], pcp[:, 0:1])
                    nc.vector.tensor_tensor(d0[:, 1:ww], pcp[:, 1:ww],
                                            pcp[:, 0:ww - 1], op=AL.subtract)
                    d1 = gp.tile([128, WIN], F32, name="d1", tag="d1",
                                 bufs=2)
                    nc.vector.tensor_tensor(d1[:, :ww], d0[:, :ww],
                                            tabfm[l - 1][:, wsl], op=AL.add)
                    d2 = gp.tile([128, WIN], F32, name="d2", tag="d2",
                                 bufs=2)
                    nc.vector.tensor_tensor(d2[:, :ww], d1[:, :ww],
                                            dinvb_t[:, wsl], op=AL.mult)
                    # Hc chunk assembly (512 = 2 windows)
                    cpos = (w * WIN) % 512
                    if cpos == 0:
                        Hcb_ch = gp.tile([128, 512], BF16, name="Hcb",
                                         tag="Hcb", bufs=2)
                        xf = gp.tile([128, 512], F32, name="xf", tag="xf",
                                     bufs=2)
                        xfb = gp.tile([128, 512], BF16, name="xfb",
                                      tag="xfb", bufs=2)
                        ch0 = w * WIN
                        cn_ = min(512, NLOC - ch0)
                        nc.sync.dma_start(xf[:, :cn_],
                                          xf_dram[:, ch0:ch0 + cn_])
                        nc.sync.dma_start(xfb[:, :cn_],
                                          xfb_dram[:, ch0:ch0 + cn_])
                    Hc = gp.tile([128, WIN], F32, name="Hc", tag="Hc",
                                 bufs=3)
                    nc.vector.tensor_scalar(Hc[:, :ww], d2[:, :ww], b_col,
                                            None, op0=AL.add)
                    nc.vector.tensor_copy(Hcb_ch[:, cpos:cpos + ww],
                                          Hc[:, :ww])
                    if cpos + ww >= 512 or w == NW2 - 1:
                        ch0 = (w * WIN + ww) - (cpos + ww)
                        cn_ = cpos + ww
                        H = gru_chunk(l, Hcb_ch, xf, xfb, cn_, gp, cps)
                        fin(ch0, cn_, H, gp, cps)

        # ---- layer 1 finish: relu, h1_out, tab2, AGs ----
        def fin1(ch0, cn_, H, gp, gps):
            Hr = gp.tile([128, 512], F32, name="Hr", tag="Hr", bufs=2)
            nc.scalar.activation(Hr[:, :cn_], H[:, :cn_], AF.Relu,
                                 bias=zero_col[:])
            H1b = gp.tile([128, 512], BF16, name="H1b", tag="H1b", bufs=2)
            nc.vector.tensor_copy(H1b[:, :cn_], Hr[:, :cn_])
            for j in range(0, cn_, 128):
                ptr = gps.tile([128, 128], F32, name="ptr", tag="ptr",
                               bufs=1)
                nc.tensor.transpose(ptr[:], Hr[:, j:j + 128],
                                    w_t["ident_f"][:])
                ob = gp.tile([128, 128], F32, name="ob", tag="ob", bufs=3)
                nc.scalar.copy(ob[:], ptr[:])
                nc.scalar.dma_start(h1_out[ch0 + j:ch0 + j + 128, :], ob[:])
                r = (ch0 + j) // 128

                def prod2(pt, rs, jj=j):
                    nc.tensor.matmul(pt[:], H1b[:, jj:jj + 128],
                                     w_t["W2T"][:], start=True, stop=True)
                tab_prod(2, r, prod2, gp, gps)
                if r == HALF_LOC // 128 - 1:
                    fire_ag(2, 0)
                if (r + 1) * 128 == NLOC:
                    fire_ag(2, 1)

        if PH >= 2:
            conv_layer(1, w_t["b1_col"][:], xp1_fm, xp1_fmb, fin1)

        # ---- layer 2 finish: skip, Hpre, BN stats ----
        def fin2(ch0, cn_, H, gp, gps):
            cs = slice(ch0, ch0 + cn_)
            pk = gps.tile([128, 512], F32, name="pk", tag="pk", bufs=1)
            nc.tensor.matmul(pk[:, :cn_], w_t["S1a"][:], nf_t[:, cs],
                             start=True, stop=False)
            nc.tensor.matmul(pk[:, :cn_], w_t["S1b"][:], te_t[:, cs],
                             start=False, stop=False)
            nc.tensor.matmul(pk[:, :cn_], w_t["skip_bias2"][:],
                             ones2[:, :cn_], start=False, stop=True)
            nc.vector.tensor_tensor(Hpre_t[:, cs], H[:, :cn_], pk[:, :cn_],
                                    op=AL.add)
            pm = gps.tile([128, 512], F32, name="pm", tag="pm", bufs=1)
            nc.tensor.matmul(pm[:, :cn_], ones2[0:1, 0:128], mask_t[:, cs],
                             start=True, stop=True)
            hm = gp.tile([128, 512], F32, name="hm", tag="hm", bufs=1)
            nc.vector.tensor_tensor(hm[:, :cn_], Hpre_t[:, cs], pm[:, :cn_],
                                    op=AL.mult)
            ci = ch0 // 512
            nc.vector.tensor_reduce(part_s[:, ci:ci + 1], hm[:, :cn_],
                                    axis=mybir.AxisListType.X, op=AL.add)
            sqs = gp.tile([128, 512], F32, name="sqs", tag="sqs", bufs=1)
            nc.scalar.activation(sqs[:, :cn_], hm[:, :cn_], AF.Square,
                                 bias=0.0, accum_out=part_q[:, ci:ci + 1])

        if PH >= 3:
            conv_layer(2, w_t["b2_col"][:], xp2_fm, xp2_fmb, fin2)
        else:
            nc.vector.memset(Hpre_t[:], 0.0)
            nc.vector.memset(part_s[:], 0.0)
            nc.vector.memset(part_q[:], 0.0)
            z1 = res.tile([128, 128], F32, name="z1")
            nc.vector.memset(z1[:], 0.0)
            if PH < 2:
                for r in range(NLOC // 128):
                    nc.scalar.dma_start(h1_out[r * 128:(r + 1) * 128, :],
                                        z1[:])

        # ================= BatchNorm finale =================
        with tc.tile_pool(name="bn", bufs=1) as bp, \
             tc.tile_pool(name="bnps", bufs=1, space="PSUM") as bps:
            nc.vector.tensor_reduce(msum[:, 0:1], part_s[:],
                                    axis=mybir.AxisListType.X, op=AL.add)
            nc.vector.tensor_reduce(msum[:, 1:2], part_q[:],
                                    axis=mybir.AxisListType.X, op=AL.add)
            nc.scalar.dma_start(bn_in[:], msum[:])
            nc.gpsimd.collective_compute(
                "AllReduce", AL.add, replica_groups=RG,
                ins=[bn_in[:]], outs=[bn_out[:]])
            nc.sync.dma_start(bnred[:], bn_out[:])
            nc.vector.tensor_scalar(mean_c[:], bnred[:, 0:1], 1.0 / N, None,
                                    op0=AL.mult)
            m2 = bp.tile([128, 1], F32, name="m2")
            nc.vector.tensor_tensor(m2[:], mean_c[:], mean_c[:], op=AL.mult)
            v1 = bp.tile([128, 1], F32, name="v1")
            nc.vector.tensor_scalar(v1[:], bnred[:, 1:2], 1.0 / N, None,
                                    op0=AL.mult)
            v2 = bp.tile([128, 1], F32, name="v2")
            nc.vector.tensor_tensor(v2[:], v1[:], m2[:], op=AL.subtract)
            v3 = bp.tile([128, 1], F32, name="v3")
            nc.vector.tensor_scalar(v3[:], v2[:], 1e-5, None, op0=AL.add)
            v4 = bp.tile([128, 1], F32, name="v4")
            nc.scalar.activation(v4[:], v3[:], AF.Sqrt, bias=zero_col[:])
            nc.vector.reciprocal(istd_c[:], v4[:])
            for r in range(NLOC // 128):
                rs = slice(r * 128, (r + 1) * 128)
                hn_ = bp.tile([128, 128], F32, name="hn_", tag="hn_", bufs=2)
                nc.vector.tensor_scalar(hn_[:], Hpre_t[:, rs],
                                        mean_c[:], istd_c[:],
                                        op0=AL.subtract, op1=AL.mult)
                ptr = bps.tile([128, 128], F32, name="ptr", tag="ptr",
                               bufs=2)
                nc.tensor.transpose(ptr[:], hn_[:], w_t["ident_f"][:])
                ob = bp.tile([128, 128], F32, name="ob", tag="ob", bufs=3)
                nc.scalar.copy(ob[:], ptr[:])
                nc.scalar.dma_start(h2_out[rs, :], ob[:])

        res_cm.__exit__(None, None, None)
    return nc


# ---------------------------------------------------------------------------
# entry point
# ---------------------------------------------------------------------------

def _install_ntff_hook():
    """Install antenv.axon_hooks (missing in this image) for trace=True."""
    import sys
    import types
    try:
        import antenv
        if getattr(antenv, "axon_hooks", None) is not None:
            return
        from trn_agent_boot.trn_boot import _ntff_profile_via_ctypes
        hook = _ntff_profile_via_ctypes("/opt/axon/libaxon_pjrt.so")
        mod = types.ModuleType("antenv.axon_hooks")
        mod.set_axon_ntff_profile_hook = lambda h: None
        mod.get_axon_ntff_profile_hook = lambda: hook
        sys.modules["antenv.axon_hooks"] = mod
        antenv.axon_hooks = mod
    except Exception:
        pass


def kernel(**inputs):
    global LAST_EXEC_NS, LAST_RESULTS
    per_core, static = _preprocess(inputs)

    nc = bacc.Bacc("TRN2", target_bir_lowering=False, debug=False,
                   num_devices=NCORES, num_swdge_queues=4)
    _build(nc, static)
    nc.compile()

    in_maps = [per_core[k] for k in range(NCORES)]
    trace = os.environ.get("KERNEL_TRACE", "0") == "1"
    if trace:
        _install_ntff_hook()
    res = run_bass_kernel_spmd(nc, in_maps, list(range(NCORES)), trace=trace)
    LAST_EXEC_NS = res.exec_time_ns
    LAST_RESULTS = res.results

    H1 = np.zeros((N, C), np.float32)
    H2 = np.zeros((N, C), np.float32)
    for k in range(NCORES):
        lo, hi_ = k * NLOC, min((k + 1) * NLOC, N)
        if lo >= N:
            break
        nrow = hi_ - lo
        H1[lo:hi_] = res.results[k]["h1_out"][:nrow]
        H2[lo:hi_] = res.results[k]["h2_out"][:nrow]
    return (H1, H2)
